# revision 36
# baseline (speedup 1.0000x reference)
"""Trainium2 Bass kernel for nn_MultiHeadAttention_77446850281793.

Reference semantics (faithful quirk: softmax over the HEADS axis):
    Qh = q @ Wq.T + bq   (per-head view)   [S, H, dk]
    scores[h, i, j] = (Qh[i,h] . Kh[j,h]) / sqrt(dk)
    attn = softmax over h (heads) of scores
    ctx[h, i] = sum_j attn[h,i,j] * Vh[j,h]
    out = concat(ctx) @ Wo.T + bo

Sharding: sequence-parallel over the 8 cores (256 query rows each).
Each core projects its own 256-row slice of q/k/v; K^T and V slices are
AllGathered (bf16) so every core holds full K/V; the head-axis softmax is
then entirely core-local. Output rows are gathered on the host.

Schedule notes (cost-model driven):
  - All matmul operands bf16: 1 cycle/row everywhere.
  - Phase A runs K -> V -> Q projections, kt-outer, paced by chunked
    weight DMAs (warm-up matmuls burn the PE p-state ramp first). The
    K and V AllGathers and all 14 remote-block readbacks are in flight
    before the attention loop starts; the readbacks interleave K/V on
    the SP ring in deadline order, with the Wo load last.
  - Attention loop per j-tile: 16 score matmuls (4-head PSUM groups,
    double buffered); exp on ACT (the pacer); head-sum tree split Pool
    (heads 0-7, starting right after exp group 1) / DVE (heads 8-15);
    normalization in 2 half-muls so the ctx matmuls can chase.
  - ctx accumulates in the swapped layout [(i-tile) i, (h,dk)]: full
    128-partition outputs, N=64 per matmul -> half the PE rows.
  - Output projection: per-bank ctx copies (ACT/DVE), 16 PE transposes
    rotated over 8 PSUM slots, N=512 matmuls against bf16 WoT, output
    copied/stored in 512-wide chunks.
"""

import numpy as np
import ml_dtypes

SEQ, DIM, HEADS, DK, NCORES = 2048, 1024, 16, 64, 8
SS = SEQ // NCORES  # 256 query rows per core
SCALE = 1.0 / 8.0  # 1/sqrt(DK); folded into Wq/bq on the host

_CACHE = {}


def _build(fake_ag=False):
    import concourse.bass as bass
    import concourse.bacc as bacc
    import concourse.tile as tile
    import concourse.mybir as mybir

    dt = mybir.dt
    f32, bf16 = dt.float32, dt.bfloat16

    nc = bacc.Bacc(
        "TRN2", target_bir_lowering=False, debug=False, num_devices=NCORES
    )

    qT = nc.dram_tensor("qT", [DIM, SS], bf16, kind="ExternalInput")
    kT = nc.dram_tensor("kT", [DIM, SS], bf16, kind="ExternalInput")
    vT = nc.dram_tensor("vT", [DIM, SS], bf16, kind="ExternalInput")
    WqT = nc.dram_tensor("WqT", [DIM, DIM], bf16, kind="ExternalInput")
    WkT = nc.dram_tensor("WkT", [DIM, DIM], bf16, kind="ExternalInput")
    WvT = nc.dram_tensor("WvT", [DIM, DIM], bf16, kind="ExternalInput")
    WoT = nc.dram_tensor("WoT", [DIM, DIM], bf16, kind="ExternalInput")
    bq = nc.dram_tensor("bq", [DIM], f32, kind="ExternalInput")
    bk = nc.dram_tensor("bk", [DIM], f32, kind="ExternalInput")
    bv = nc.dram_tensor("bv", [DIM], f32, kind="ExternalInput")
    bo = nc.dram_tensor("bo", [DIM], f32, kind="ExternalInput")
    out = nc.dram_tensor("out", [SS, DIM], f32, kind="ExternalOutput")

    with tile.TileContext(nc) as tc:
        _emit(nc, tc, bass, mybir, locals(), fake_ag=fake_ag)
    nc.compile()
    return nc


def _emit(nc, tc, bass, mybir, io, fake_ag=False):
    dt = mybir.dt
    f32, bf16 = dt.float32, dt.bfloat16
    AF = mybir.ActivationFunctionType
    qT, kT, vT = io["qT"], io["kT"], io["vT"]
    WqT, WkT, WvT, WoT = io["WqT"], io["WkT"], io["WvT"], io["WoT"]
    bq, bk, bv, bo = io["bq"], io["bk"], io["bv"], io["bo"]
    out = io["out"]

    # head h -> column slot in the per-j-tile score/exp buffers (the two
    # heads of a row-packed matmul pair go to different PSUM banks).
    def slot_col(h):
        g, u, par = h // 4, (h % 4) // 2, h % 2
        slot = u if par == 0 else 2 + u
        return g * 4 * SS + slot * SS

    with (
        tc.tile_pool(name="constp", bufs=1) as constp,
        tc.tile_pool(name="qhtp", bufs=1) as qhtp,
        tc.tile_pool(name="kvp", bufs=1) as kvp,
        tc.tile_pool(name="dramp", bufs=1, space="DRAM") as dramp,
    ):
        ones = constp.tile([1, 128], bf16)
        nc.gpsimd.memset(ones[:], 1.0)
        zb = constp.tile([128, 1], f32)
        nc.gpsimd.memset(zb[:], 0.0)
        z512 = constp.tile([1, 512], bf16)
        nc.gpsimd.memset(z512[:], 0.0)
        ident = constp.tile([128, 128], bf16)
        from concourse.masks import make_identity
        make_identity(nc, ident[:])

        bq_sb = constp.tile([128, 8], f32)
        bk_sb = constp.tile([128, 8], f32)
        bv_sb = constp.tile([1, DIM], bf16)
        bo_sb = constp.tile([1, DIM], bf16)

        aspace = "Local" if fake_ag else "Shared"
        ag_in_k = dramp.tile([DIM, SS], bf16)
        ag_in_v = dramp.tile([DIM, SS], bf16)
        ag_out_k = dramp.tile([NCORES * DIM, SS], bf16, addr_space=aspace)
        ag_out_v = dramp.tile([NCORES * DIM, SS], bf16, addr_space=aspace)

        QhT_sb = qhtp.tile([128, 8 * SS], bf16)
        KhT_c2 = qhtp.tile([128, 8 * SS], bf16)

        # long-lived attention operands
        KhT_sb = kvp.tile([128, 8 * SEQ], bf16)
        Vh_sb = kvp.tile([128, 16 * DIM], bf16)
        WoT_sb = kvp.tile([128, 8 * DIM], bf16)
        KhT_v = KhT_sb[:].rearrange("p (t j) -> p t j", t=8)
        Vh_v = Vh_sb[:].rearrange("p (jt d) -> p jt d", jt=16)

        pid = nc.partition_id()

        def load_w(pool, dram_w, name, nchunks=8):
            w_sb = pool.tile([128, 8 * DIM], bf16, name=name)
            src = dram_w.ap().rearrange("(t p) d -> p t d", p=128)
            dst = w_sb[:].rearrange("p (t d) -> p t d", t=8)
            step = 8 // nchunks
            for h in range(nchunks):
                nc.sync.dma_start(
                    dst[:, step * h : step * (h + 1), :],
                    src[:, step * h : step * (h + 1), :],
                )
            return w_sb

        def load_x(pool, dram_x, name):
            x_sb = pool.tile([128, 8 * SS], bf16, name=name)
            nc.sync.dma_start(
                x_sb[:].rearrange("p (t j) -> p t j", t=8),
                dram_x.ap().rearrange("(t p) j -> p t j", p=128),
            )
            return x_sb

        # kt-outer projection: 8 half-bank accumulators in 4 PSUM tiles;
        # matmuls for chunk kt start as soon as that chunk of W lands and
        # the PSUM->SBUF bias-copies (alternating DVE/ACT) chase the last
        # chunk's per-mt stops.
        def project_T(psA, x_sb, w_sb, bias_sb, dst_sb, tag):
            # one accumulation group per PSUM bank: start=True clears the
            # whole 2KB bank on HW, so mt-groups may NOT share banks
            ps = [
                psA.tile([128, 512], f32, tag=f"{tag}{m}", name=f"ps_{tag}{m}")
                for m in range(8)
            ]
            for kt in range(8):
                for mt in range(8):
                    nc.tensor.matmul(
                        ps[mt][:, 0:SS],
                        w_sb[:, kt * DIM + mt * 128 : kt * DIM + (mt + 1) * 128],
                        x_sb[:, kt * SS : (kt + 1) * SS],
                        start=(kt == 0), stop=(kt == 7),
                    )
            for mt in range(8):
                src = ps[mt][:, 0:SS]
                dst = dst_sb[:, mt * SS : (mt + 1) * SS]
                if mt % 2 == 0:
                    nc.vector.tensor_scalar_add(dst, src, bias_sb[:, mt : mt + 1])
                else:
                    nc.scalar.activation(dst, src, AF.Identity,
                                         bias=bias_sb[:, mt : mt + 1], scale=1.0)

        # ---------------- Phase A: K, V, Q projections ----------------------
        with (
            tc.tile_pool(name="wpk", bufs=1) as wpk,
            tc.tile_pool(name="wpvq", bufs=1) as wpvq,
            tc.tile_pool(name="psA", bufs=1, space="PSUM") as psA,
        ):
            kT_sb = load_x(wpk, kT, "kT_sb")
            WkT_sb = load_w(wpk, WkT, "WkT_sb")
            nc.sync.dma_start(bk_sb[:], bk.ap().rearrange("(t p) -> p t", p=128))
            nc.sync.dma_start(bq_sb[:], bq.ap().rearrange("(t p) -> p t", p=128))
            # casting (f32 -> bf16) bias DMAs ride the SWDGE ring
            nc.gpsimd.dma_start(bv_sb[:], bv.ap().unsqueeze(0))
            nc.gpsimd.dma_start(bo_sb[:], bo.ap().unsqueeze(0))
            vT_sb = load_x(wpvq, vT, "vT_sb")
            WvT_sb = load_w(wpvq, WvT, "WvT_sb")
            qT_sb = load_x(wpvq, qT, "qT_sb")
            WqT_sb = load_w(wpvq, WqT, "WqT_sb")

            # p-state warm-up while the first weight chunks stream in
            # (borrows the first projection bank; WAW keeps it ordered)
            warm = psA.tile([128, 512], f32, tag="kq0", name="warm")
            for _ in range(24):
                nc.tensor.matmul(warm[0:64, 0:64], ident[:, 0:64], ident[:, 0:64],
                                 start=True, stop=True)

            # K projection + AllGather staging (SWDGE ring)
            project_T(psA, kT_sb, WkT_sb, bk_sb, KhT_c2, "kq")
            nc.gpsimd.dma_start(
                ag_in_k[:, :].rearrange("(t p) j -> p t j", p=128),
                KhT_c2[:].rearrange("p (t j) -> p t j", t=8),
            )
            # own K block: SBUF->SBUF on DVE, early (DVE is idle here)
            nc.vector.tensor_copy(KhT_v[:, :, 0:SS],
                                  KhT_c2[:].rearrange("p (t j) -> p t j", t=8))
            if fake_ag:
                nc.gpsimd.dma_start(
                    ag_out_k[:, :].rearrange("(c r) j -> c r j", c=NCORES)[0],
                    ag_in_k[:, :])
            else:
                nc.gpsimd.collective_compute(
                    "AllGather", mybir.AluOpType.bypass,
                    replica_groups=[list(range(NCORES))],
                    ins=[ag_in_k[:, :]], outs=[ag_out_k[:, :]],
                )

            # V projection straight into the own-block slots of Vh_sb
            # (copies on the otherwise idle ACT)
            for st in range(2):
                for nh in range(2):
                    vps = psA.tile([128, 512], f32, tag=f"kq{2*st+nh}",
                                   name="vps")
                    for kt in range(8):
                        nc.tensor.matmul(
                            vps[:],
                            vT_sb[:, kt * SS + st * 128 : kt * SS + (st + 1) * 128],
                            WvT_sb[:, kt * DIM + nh * 512 : kt * DIM + (nh + 1) * 512],
                            start=(kt == 0), stop=False,
                        )
                    nc.tensor.matmul(
                        vps[:], ones[:, 0:128],
                        bv_sb[:, nh * 512 : (nh + 1) * 512],
                        start=False, stop=True,
                    )
                    nc.scalar.activation(
                        Vh_sb[:, st * DIM + nh * 512 : st * DIM + (nh + 1) * 512],
                        vps[:], AF.Copy,
                    )

            # Q projection (scale pre-folded into WqT/bq on host)
            project_T(psA, qT_sb, WqT_sb, bq_sb, QhT_sb, "kq")

            # V AllGather chain + all readbacks, interleaved in deadline
            # order on the SP ring (ring order = emission order); by the
            # time SP reaches ag_in_v its inputs are long done.
            nc.sync.dma_start(
                ag_in_v[:, :].rearrange("(a p c) j -> p a (c j)", a=2, p=128),
                Vh_sb[:, 0 : 2 * DIM].rearrange("p (a d) -> p a d", a=2),
            )
            if fake_ag:
                nc.sync.dma_start(
                    ag_out_v[:, :].rearrange("(c r) j -> c r j", c=NCORES)[0],
                    ag_in_v[:, :])
            else:
                nc.gpsimd.collective_compute(
                    "AllGather", mybir.AluOpType.bypass,
                    replica_groups=[list(range(NCORES))],
                    ins=[ag_in_v[:, :]], outs=[ag_out_v[:, :]],
                )

            def emit_rbk(s):
                blk = (pid + s) % NCORES
                nc.sync.dma_start(
                    KhT_v[:, :, SS * s : SS * (s + 1)],
                    ag_out_k[bass.ds(blk * DIM, DIM), :].rearrange(
                        "(t p) j -> p t j", p=128),
                )

            def emit_rbv(s):
                blk = (pid + s) % NCORES
                nc.sync.dma_start(
                    Vh_v[:, 2 * s : 2 * s + 2, :],
                    ag_out_v[bass.ds(blk * DIM, DIM), :].rearrange(
                        "(a p c2) j -> p a (c2 j)", a=2, p=128),
                )

            emit_rbk(1)
            emit_rbk(2)
            emit_rbv(1)
            emit_rbk(3)
            emit_rbv(2)
            for s in range(4, NCORES):
                emit_rbk(s)
                emit_rbv(s - 1)
            emit_rbv(7)
            # Wo load trails everything on the SP ring
            wo_src = WoT.ap().rearrange("(t p) d -> p t d", p=128)
            wo_dst = WoT_sb[:].rearrange("p (t d) -> p t d", t=8)
            for h in range(2):
                nc.sync.dma_start(wo_dst[:, 4 * h : 4 * h + 4, :],
                                  wo_src[:, 4 * h : 4 * h + 4, :])

        # ---------------- Phase B: attention over full K/V ------------------
        with (
            tc.tile_pool(name="attnp", bufs=2) as attnp,
            tc.tile_pool(name="psB", bufs=1, space="PSUM") as psB,
        ):
            attn_q = []
            sc_last = {}
            from concourse.tile import add_dep_helper

            ctx_ps = None  # allocated after j-tile 0's emission

            def emit_ctx_half(jt, attn, half):
                for h in range(8 * half, 8 * half + 8):
                    for it in range(2):
                        is_stop = jt == 15 and (h % 8) == 7
                        mm = nc.tensor.matmul(
                            ctx_ps[:, it * DIM + h * DK : it * DIM + (h + 1) * DK],
                            attn[:, slot_col(h) + it * 128 : slot_col(h) + (it + 1) * 128],
                            Vh_sb[:, jt * DIM + h * DK : jt * DIM + (h + 1) * DK],
                            start=False, stop=is_stop,
                            skip_group_check=True,
                        )
                        if h % 8 == 0 and it == 0 and (jt + 1) in sc_last:
                            add_dep_helper(
                                mm.ins, sc_last[jt + 1].ins, sync=False,
                                reason="scores ahead of ctx on PE",
                            )

            def emit_jt(jt):
                e_sb = attnp.tile([128, 16 * SS], bf16, tag="e", bufs=3)
                for g in range(4):
                    sc_ps = psB.tile([128, 4 * SS], f32, tag="sc", bufs=2)
                    for u in range(2):
                        for par in range(2):
                            h = 4 * g + 2 * u + par
                            t = h // 2
                            sc_last[jt] = nc.tensor.matmul(
                                sc_ps[:, (u if par == 0 else 2 + u) * SS :][:, :SS],
                                KhT_sb[64 * par : 64 * par + 64,
                                       t * SEQ + jt * 128 : t * SEQ + (jt + 1) * 128],
                                QhT_sb[64 * par : 64 * par + 64,
                                       t * SS : (t + 1) * SS],
                                start=True, stop=True,
                            )
                    nc.scalar.activation(
                        e_sb[:, g * 4 * SS : (g + 1) * 4 * SS], sc_ps[:],
                        AF.Exp, bias=zb[:],
                    )
                    # Pool tree (heads 0-7) starts right after exp group 1
                    if g == 1:
                        a1 = attnp.tile([128, 4 * SS], bf16, tag="a1", bufs=2)
                        nc.gpsimd.tensor_add(a1[:], e_sb[:, 0 : 4 * SS],
                                             e_sb[:, 4 * SS : 8 * SS])
                        a2 = attnp.tile([128, 2 * SS], bf16, tag="a2", bufs=2)
                        nc.gpsimd.tensor_add(a2[:], a1[:, 0 : 2 * SS],
                                             a1[:, 2 * SS : 4 * SS])
                        a3 = attnp.tile([128, SS], bf16, tag="a3", bufs=2)
                        nc.gpsimd.tensor_add(a3[:], a2[:, 0:SS], a2[:, SS : 2 * SS])
                # DVE tree (heads 8-15), combine, recip, normalize (2 halves)
                b1 = attnp.tile([128, 4 * SS], bf16, tag="b1", bufs=2)
                nc.vector.tensor_add(b1[:], e_sb[:, 8 * SS : 12 * SS],
                                     e_sb[:, 12 * SS : 16 * SS])
                b2 = attnp.tile([128, 2 * SS], bf16, tag="b2", bufs=2)
                nc.vector.tensor_add(b2[:], b1[:, 0 : 2 * SS], b1[:, 2 * SS : 4 * SS])
                b3 = attnp.tile([128, SS], bf16, tag="b3", bufs=2)
                nc.vector.tensor_add(b3[:], b2[:, 0:SS], b2[:, SS : 2 * SS])
                Dsum = attnp.tile([128, SS], f32, tag="Dsum", bufs=2)
                nc.vector.tensor_add(Dsum[:], a3[:], b3[:])
                Rf = attnp.tile([128, SS], f32, tag="Rf", bufs=2)
                nc.vector.reciprocal_approx_fast(Rf[:], Dsum[:])
                Rcp = attnp.tile([128, SS], bf16, tag="Rcp", bufs=2)
                nc.vector.tensor_copy(Rcp[:], Rf[:])
                attn = attnp.tile([128, 16 * SS], bf16, tag="attn", bufs=3)
                for half in range(2):
                    nc.vector.tensor_mul(
                        attn[:, half * 8 * SS : (half + 1) * 8 * SS].rearrange(
                            "p (s j) -> p s j", s=8),
                        e_sb[:, half * 8 * SS : (half + 1) * 8 * SS].rearrange(
                            "p (s j) -> p s j", s=8),
                        Rcp[:].unsqueeze(1).broadcast_to([128, 8, SS]),
                    )
                attn_q.append((jt, attn))
                if len(attn_q) > 2:
                    j0, a0 = attn_q.pop(0)
                    emit_ctx_half(j0, a0, 0)
                    emit_ctx_half(j0, a0, 1)

            emit_jt(0)

            # ctx accumulator: zero the 4 banks via one full-bank
            # start=True matmul each (runs in early-loop PE slack).
            ctx_ps = psB.tile([128, 2 * DIM], f32, tag="ctx", name="ctx_ps")
            for b in range(4):
                nc.tensor.matmul(
                    ctx_ps[:, 512 * b : 512 * (b + 1)],
                    z512[:, 0:128], z512[:, 0:512],
                    start=True, stop=False, skip_group_check=True,
                )

            for jt in range(1, 16):
                emit_jt(jt)
            while attn_q:
                j0, a0 = attn_q.pop(0)
                emit_ctx_half(j0, a0, 0)
                emit_ctx_half(j0, a0, 1)

            # keep the PE p-state warm through the softmax-flush idle so
            # the output projection charges full-speed cycles
            warm2 = psB.tile([128, 4 * SS], f32, tag="sc", bufs=2, name="warm2")
            for _ in range(56):
                nc.tensor.matmul(warm2[0:64, 0:64], ident[:, 0:64],
                                 ident[:, 0:64], start=True, stop=True)

            # ---------------- Phase C: output projection --------------------
            with tc.tile_pool(name="cpool", bufs=1) as cpool:
                ctx_sb = cpool.tile([128, 2 * DIM], bf16)
                ctxT_sb = cpool.tile([128, 2 * DIM], bf16)
                # per-bank ctx copies (split ACT/DVE), each chased by a
                # 512-wide xbar DMA transpose into the d-tile layout
                # ctxT[(dt) d, it*128 + i]. No PE or PSUM involved.
                ctxT_v = ctxT_sb[:].rearrange(
                    "p (dt itt i) -> p dt itt i", dt=8, itt=2)
                for idx, (it, bh) in enumerate(
                        ((0, 0), (1, 0), (0, 1), (1, 1))):
                    b = it * 2 + bh
                    src = ctx_ps[:, b * 512 : (b + 1) * 512]
                    dst = ctx_sb[:, b * 512 : (b + 1) * 512]
                    if idx % 2 == 0:
                        nc.scalar.activation(dst, src, AF.Copy)
                    else:
                        nc.vector.tensor_copy(dst, src)
                    nc.sync.dma_start_transpose(
                        ctxT_v[:, 4 * bh : 4 * bh + 4, it, :], dst)
                # O = ctx @ Wo^T + bo : out[(it) i, n], N=512 per matmul;
                # output copied (ACT) and stored in 512-wide chunks on
                # alternating DMA rings (SP / ACT).
                out_sb = cpool.tile([128, 2 * DIM], f32)
                ops_both = psB.tile([128, 2 * DIM], f32, tag="ctx", name="ops")
                for it in range(2):
                    ops = ops_both[:, it * DIM : (it + 1) * DIM]
                    for nh in range(2):
                        for kt in range(8):
                            nc.tensor.matmul(
                                ops[:, nh * 512 : (nh + 1) * 512],
                                ctxT_sb[:, kt * SS + it * 128 : kt * SS + (it + 1) * 128],
                                WoT_sb[:, kt * DIM + nh * 512 : kt * DIM + (nh + 1) * 512],
                                start=(kt == 0), stop=False,
                                skip_group_check=True,
                            )
                        nc.tensor.matmul(
                            ops[:, nh * 512 : (nh + 1) * 512], ones[:, 0:128],
                            bo_sb[:, nh * 512 : (nh + 1) * 512],
                            start=False, stop=True,
                            skip_group_check=True,
                        )
                        osl = slice(it * DIM + nh * 512, it * DIM + (nh + 1) * 512)
                        nc.scalar.activation(
                            out_sb[:, osl], ops[:, nh * 512 : (nh + 1) * 512],
                            AF.Copy,
                        )
                        dma_eng = nc.sync if (2 * it + nh) % 2 == 0 else nc.scalar
                        dma_eng.dma_start(
                            out.ap().rearrange(
                                "(mt p) (nh d) -> p mt nh d", p=128, nh=2
                            )[:, it, nh, :],
                            out_sb[:, osl],
                        )


def get_nc():
    if "nc" not in _CACHE:
        _CACHE["nc"] = _build()
    return _CACHE["nc"]


def make_in_maps(inputs):
    f = lambda x: np.ascontiguousarray(np.asarray(x, dtype=np.float32))
    bf = ml_dtypes.bfloat16
    q, k, v = f(inputs["q"]), f(inputs["k"]), f(inputs["v"])
    WqTs = np.ascontiguousarray((f(inputs["Wq"]) * SCALE).T.astype(bf))
    WkT = np.ascontiguousarray(f(inputs["Wk"]).T.astype(bf))
    WvT = np.ascontiguousarray(f(inputs["Wv"]).T.astype(bf))
    WoT = np.ascontiguousarray(f(inputs["Wo"]).T.astype(bf))
    bqs = f(inputs["bq"]) * np.float32(SCALE)
    bk, bv, bo = f(inputs["bk"]), f(inputs["bv"]), f(inputs["bo"])
    in_maps = []
    for c in range(NCORES):
        sl = slice(c * SS, (c + 1) * SS)
        in_maps.append({
            "qT": np.ascontiguousarray(q[sl].T.astype(bf)),
            "kT": np.ascontiguousarray(k[sl].T.astype(bf)),
            "vT": np.ascontiguousarray(v[sl].T.astype(bf)),
            "WqT": WqTs, "WkT": WkT, "WvT": WvT, "WoT": WoT,
            "bq": bqs, "bk": bk, "bv": bv, "bo": bo,
        })
    return in_maps


def run(inputs, **kwargs):
    """Run on hardware; returns (output, BassKernelResults)."""
    from concourse import bass_utils

    nc = get_nc()
    res = bass_utils.run_bass_kernel_spmd(
        nc, make_in_maps(inputs), core_ids=list(range(NCORES)), **kwargs
    )
    rows = [res.results[c]["out"] for c in range(NCORES)]
    full = np.concatenate(rows, axis=0).astype(np.float32)
    return full.reshape(1, SEQ, DIM), res


def kernel(**inputs) -> np.ndarray:
    out, _ = run(inputs)
    return out


# revision 37
# speedup vs baseline: 1.0065x; 1.0065x over previous
"""Trainium2 Bass kernel for nn_MultiHeadAttention_77446850281793.

Reference semantics (faithful quirk: softmax over the HEADS axis):
    Qh = q @ Wq.T + bq   (per-head view)   [S, H, dk]
    scores[h, i, j] = (Qh[i,h] . Kh[j,h]) / sqrt(dk)
    attn = softmax over h (heads) of scores
    ctx[h, i] = sum_j attn[h,i,j] * Vh[j,h]
    out = concat(ctx) @ Wo.T + bo

Sharding: sequence-parallel over the 8 cores (256 query rows each).
Each core projects its own 256-row slice of q/k/v; K^T and V slices are
AllGathered (bf16) so every core holds full K/V; the head-axis softmax is
then entirely core-local. Output rows are gathered on the host.

Schedule notes (cost-model driven):
  - All matmul operands bf16: 1 cycle/row everywhere.
  - Phase A runs K -> V -> Q projections, kt-outer, paced by chunked
    weight DMAs (warm-up matmuls burn the PE p-state ramp first). The
    K and V AllGathers and all 14 remote-block readbacks are in flight
    before the attention loop starts; the readbacks interleave K/V on
    the SP ring in deadline order, with the Wo load last.
  - Attention loop per j-tile: 16 score matmuls (4-head PSUM groups,
    double buffered); exp on ACT (the pacer); head-sum tree split Pool
    (heads 0-7, starting right after exp group 1) / DVE (heads 8-15);
    normalization in 2 half-muls so the ctx matmuls can chase.
  - ctx accumulates in the swapped layout [(i-tile) i, (h,dk)]: full
    128-partition outputs, N=64 per matmul -> half the PE rows.
  - Output projection: per-bank ctx copies (ACT/DVE), 16 PE transposes
    rotated over 8 PSUM slots, N=512 matmuls against bf16 WoT, output
    copied/stored in 512-wide chunks.
"""

import numpy as np
import ml_dtypes

SEQ, DIM, HEADS, DK, NCORES = 2048, 1024, 16, 64, 8
SS = SEQ // NCORES  # 256 query rows per core
SCALE = 1.0 / 8.0  # 1/sqrt(DK); folded into Wq/bq on the host

_CACHE = {}


def _build(fake_ag=False):
    import concourse.bass as bass
    import concourse.bacc as bacc
    import concourse.tile as tile
    import concourse.mybir as mybir

    dt = mybir.dt
    f32, bf16 = dt.float32, dt.bfloat16

    nc = bacc.Bacc(
        "TRN2", target_bir_lowering=False, debug=False, num_devices=NCORES
    )

    qT = nc.dram_tensor("qT", [DIM, SS], bf16, kind="ExternalInput")
    kT = nc.dram_tensor("kT", [DIM, SS], bf16, kind="ExternalInput")
    vT = nc.dram_tensor("vT", [DIM, SS], bf16, kind="ExternalInput")
    WqT = nc.dram_tensor("WqT", [DIM, DIM], bf16, kind="ExternalInput")
    WkT = nc.dram_tensor("WkT", [DIM, DIM], bf16, kind="ExternalInput")
    WvT = nc.dram_tensor("WvT", [DIM, DIM], bf16, kind="ExternalInput")
    WoT = nc.dram_tensor("WoT", [DIM, DIM], bf16, kind="ExternalInput")
    bq = nc.dram_tensor("bq", [DIM], f32, kind="ExternalInput")
    bk = nc.dram_tensor("bk", [DIM], f32, kind="ExternalInput")
    bv = nc.dram_tensor("bv", [DIM], f32, kind="ExternalInput")
    bo = nc.dram_tensor("bo", [DIM], f32, kind="ExternalInput")
    out = nc.dram_tensor("out", [SS, DIM], f32, kind="ExternalOutput")

    with tile.TileContext(nc) as tc:
        _emit(nc, tc, bass, mybir, locals(), fake_ag=fake_ag)
    nc.compile()
    return nc


def _emit(nc, tc, bass, mybir, io, fake_ag=False):
    dt = mybir.dt
    f32, bf16 = dt.float32, dt.bfloat16
    AF = mybir.ActivationFunctionType
    qT, kT, vT = io["qT"], io["kT"], io["vT"]
    WqT, WkT, WvT, WoT = io["WqT"], io["WkT"], io["WvT"], io["WoT"]
    bq, bk, bv, bo = io["bq"], io["bk"], io["bv"], io["bo"]
    out = io["out"]

    # head h -> column slot in the per-j-tile score/exp buffers (the two
    # heads of a row-packed matmul pair go to different PSUM banks).
    def slot_col(h):
        g, u, par = h // 4, (h % 4) // 2, h % 2
        slot = u if par == 0 else 2 + u
        return g * 4 * SS + slot * SS

    with (
        tc.tile_pool(name="constp", bufs=1) as constp,
        tc.tile_pool(name="qhtp", bufs=1) as qhtp,
        tc.tile_pool(name="kvp", bufs=1) as kvp,
        tc.tile_pool(name="dramp", bufs=1, space="DRAM") as dramp,
    ):
        ones = constp.tile([1, 128], bf16)
        nc.gpsimd.memset(ones[:], 1.0)
        zb = constp.tile([128, 1], f32)
        nc.gpsimd.memset(zb[:], 0.0)
        z512 = constp.tile([1, 512], bf16)
        nc.gpsimd.memset(z512[:], 0.0)
        ident = constp.tile([128, 128], bf16)
        from concourse.masks import make_identity
        make_identity(nc, ident[:])

        bq_sb = constp.tile([128, 8], f32)
        bk_sb = constp.tile([128, 8], f32)
        bv_sb = constp.tile([1, DIM], bf16)
        bo_sb = constp.tile([1, DIM], bf16)

        aspace = "Local" if fake_ag else "Shared"
        ag_in_k = dramp.tile([DIM, SS], bf16)
        ag_in_v = dramp.tile([DIM, SS], bf16)
        ag_out_k = dramp.tile([NCORES * DIM, SS], bf16, addr_space=aspace)
        ag_out_v = dramp.tile([NCORES * DIM, SS], bf16, addr_space=aspace)

        QhT_sb = qhtp.tile([128, 8 * SS], bf16)
        KhT_c2 = qhtp.tile([128, 8 * SS], bf16)

        # long-lived attention operands
        KhT_sb = kvp.tile([128, 8 * SEQ], bf16)
        Vh_sb = kvp.tile([128, 16 * DIM], bf16)
        WoT_sb = kvp.tile([128, 8 * DIM], bf16)
        KhT_v = KhT_sb[:].rearrange("p (t j) -> p t j", t=8)
        Vh_v = Vh_sb[:].rearrange("p (jt d) -> p jt d", jt=16)

        pid = nc.partition_id()

        def load_w(pool, dram_w, name, nchunks=8):
            w_sb = pool.tile([128, 8 * DIM], bf16, name=name)
            src = dram_w.ap().rearrange("(t p) d -> p t d", p=128)
            dst = w_sb[:].rearrange("p (t d) -> p t d", t=8)
            step = 8 // nchunks
            for h in range(nchunks):
                nc.sync.dma_start(
                    dst[:, step * h : step * (h + 1), :],
                    src[:, step * h : step * (h + 1), :],
                )
            return w_sb

        def load_x(pool, dram_x, name):
            x_sb = pool.tile([128, 8 * SS], bf16, name=name)
            nc.sync.dma_start(
                x_sb[:].rearrange("p (t j) -> p t j", t=8),
                dram_x.ap().rearrange("(t p) j -> p t j", p=128),
            )
            return x_sb

        # kt-outer projection: 8 half-bank accumulators in 4 PSUM tiles;
        # matmuls for chunk kt start as soon as that chunk of W lands and
        # the PSUM->SBUF bias-copies (alternating DVE/ACT) chase the last
        # chunk's per-mt stops.
        def project_T(psA, x_sb, w_sb, bias_sb, dst_sb, tag):
            # one accumulation group per PSUM bank: start=True clears the
            # whole 2KB bank on HW, so mt-groups may NOT share banks
            ps = [
                psA.tile([128, 512], f32, tag=f"{tag}{m}", name=f"ps_{tag}{m}")
                for m in range(8)
            ]
            for kt in range(8):
                for mt in range(8):
                    nc.tensor.matmul(
                        ps[mt][:, 0:SS],
                        w_sb[:, kt * DIM + mt * 128 : kt * DIM + (mt + 1) * 128],
                        x_sb[:, kt * SS : (kt + 1) * SS],
                        start=(kt == 0), stop=(kt == 7),
                    )
            for mt in range(8):
                src = ps[mt][:, 0:SS]
                dst = dst_sb[:, mt * SS : (mt + 1) * SS]
                if mt % 2 == 0:
                    nc.vector.tensor_scalar_add(dst, src, bias_sb[:, mt : mt + 1])
                else:
                    nc.scalar.activation(dst, src, AF.Identity,
                                         bias=bias_sb[:, mt : mt + 1], scale=1.0)

        # ---------------- Phase A: K, V, Q projections ----------------------
        with (
            tc.tile_pool(name="wpk", bufs=1) as wpk,
            tc.tile_pool(name="wpvq", bufs=1) as wpvq,
            tc.tile_pool(name="psA", bufs=1, space="PSUM") as psA,
        ):
            kT_sb = load_x(wpk, kT, "kT_sb")
            WkT_sb = load_w(wpk, WkT, "WkT_sb")
            nc.sync.dma_start(bk_sb[:], bk.ap().rearrange("(t p) -> p t", p=128))
            nc.sync.dma_start(bq_sb[:], bq.ap().rearrange("(t p) -> p t", p=128))
            # casting (f32 -> bf16) bias DMAs ride the SWDGE ring
            nc.gpsimd.dma_start(bv_sb[:], bv.ap().unsqueeze(0))
            nc.gpsimd.dma_start(bo_sb[:], bo.ap().unsqueeze(0))
            vT_sb = load_x(wpvq, vT, "vT_sb")
            WvT_sb = load_w(wpvq, WvT, "WvT_sb")
            qT_sb = load_x(wpvq, qT, "qT_sb")
            WqT_sb = load_w(wpvq, WqT, "WqT_sb")

            # p-state warm-up while the first weight chunks stream in
            # (borrows the first projection bank; WAW keeps it ordered)
            warm = psA.tile([128, 512], f32, tag="kq0", name="warm")
            for _ in range(24):
                nc.tensor.matmul(warm[0:64, 0:64], ident[:, 0:64], ident[:, 0:64],
                                 start=True, stop=True)

            # K projection + AllGather staging (SWDGE ring)
            project_T(psA, kT_sb, WkT_sb, bk_sb, KhT_c2, "kq")
            nc.gpsimd.dma_start(
                ag_in_k[:, :].rearrange("(t p) j -> p t j", p=128),
                KhT_c2[:].rearrange("p (t j) -> p t j", t=8),
            )
            # own K block: SBUF->SBUF on DVE, early (DVE is idle here)
            nc.vector.tensor_copy(KhT_v[:, :, 0:SS],
                                  KhT_c2[:].rearrange("p (t j) -> p t j", t=8))
            if fake_ag:
                nc.gpsimd.dma_start(
                    ag_out_k[:, :].rearrange("(c r) j -> c r j", c=NCORES)[0],
                    ag_in_k[:, :])
            else:
                nc.gpsimd.collective_compute(
                    "AllGather", mybir.AluOpType.bypass,
                    replica_groups=[list(range(NCORES))],
                    ins=[ag_in_k[:, :]], outs=[ag_out_k[:, :]],
                )

            # V projection straight into the own-block slots of Vh_sb
            # (copies on the otherwise idle ACT)
            for st in range(2):
                for nh in range(2):
                    vps = psA.tile([128, 512], f32, tag=f"kq{2*st+nh}",
                                   name="vps")
                    for kt in range(8):
                        nc.tensor.matmul(
                            vps[:],
                            vT_sb[:, kt * SS + st * 128 : kt * SS + (st + 1) * 128],
                            WvT_sb[:, kt * DIM + nh * 512 : kt * DIM + (nh + 1) * 512],
                            start=(kt == 0), stop=False,
                        )
                    nc.tensor.matmul(
                        vps[:], ones[:, 0:128],
                        bv_sb[:, nh * 512 : (nh + 1) * 512],
                        start=False, stop=True,
                    )
                    nc.scalar.activation(
                        Vh_sb[:, st * DIM + nh * 512 : st * DIM + (nh + 1) * 512],
                        vps[:], AF.Copy,
                    )

            # Q projection (scale pre-folded into WqT/bq on host)
            project_T(psA, qT_sb, WqT_sb, bq_sb, QhT_sb, "kq")

            # V AllGather chain + all readbacks, interleaved in deadline
            # order on the SP ring (ring order = emission order); by the
            # time SP reaches ag_in_v its inputs are long done.
            nc.sync.dma_start(
                ag_in_v[:, :].rearrange("(a p c) j -> p a (c j)", a=2, p=128),
                Vh_sb[:, 0 : 2 * DIM].rearrange("p (a d) -> p a d", a=2),
            )
            if fake_ag:
                nc.sync.dma_start(
                    ag_out_v[:, :].rearrange("(c r) j -> c r j", c=NCORES)[0],
                    ag_in_v[:, :])
            else:
                nc.gpsimd.collective_compute(
                    "AllGather", mybir.AluOpType.bypass,
                    replica_groups=[list(range(NCORES))],
                    ins=[ag_in_v[:, :]], outs=[ag_out_v[:, :]],
                )

            def emit_rbk(s):
                blk = (pid + s) % NCORES
                nc.sync.dma_start(
                    KhT_v[:, :, SS * s : SS * (s + 1)],
                    ag_out_k[bass.ds(blk * DIM, DIM), :].rearrange(
                        "(t p) j -> p t j", p=128),
                )

            def emit_rbv(s):
                blk = (pid + s) % NCORES
                nc.sync.dma_start(
                    Vh_v[:, 2 * s : 2 * s + 2, :],
                    ag_out_v[bass.ds(blk * DIM, DIM), :].rearrange(
                        "(a p c2) j -> p a (c2 j)", a=2, p=128),
                )

            emit_rbk(1)
            emit_rbk(2)
            emit_rbv(1)
            emit_rbk(3)
            emit_rbv(2)
            for s in range(4, NCORES):
                emit_rbk(s)
                emit_rbv(s - 1)
            emit_rbv(7)
            # Wo load trails everything on the SP ring
            wo_src = WoT.ap().rearrange("(t p) d -> p t d", p=128)
            wo_dst = WoT_sb[:].rearrange("p (t d) -> p t d", t=8)
            for h in range(2):
                nc.sync.dma_start(wo_dst[:, 4 * h : 4 * h + 4, :],
                                  wo_src[:, 4 * h : 4 * h + 4, :])

        # ---------------- Phase B: attention over full K/V ------------------
        with (
            tc.tile_pool(name="attnp", bufs=2) as attnp,
            tc.tile_pool(name="psB", bufs=1, space="PSUM") as psB,
        ):
            attn_q = []
            sc_last = {}
            from concourse.tile import add_dep_helper

            ctx_ps = None  # allocated after j-tile 0's emission

            def emit_ctx_half(jt, attn, half):
                for h in range(8 * half, 8 * half + 8):
                    for it in range(2):
                        is_stop = jt == 15 and (h % 8) == 7
                        mm = nc.tensor.matmul(
                            ctx_ps[:, it * DIM + h * DK : it * DIM + (h + 1) * DK],
                            attn[:, slot_col(h) + it * 128 : slot_col(h) + (it + 1) * 128],
                            Vh_sb[:, jt * DIM + h * DK : jt * DIM + (h + 1) * DK],
                            start=False, stop=is_stop,
                            skip_group_check=True,
                        )
                        if h % 8 == 0 and it == 0 and (jt + 1) in sc_last:
                            add_dep_helper(
                                mm.ins, sc_last[jt + 1].ins, sync=False,
                                reason="scores ahead of ctx on PE",
                            )

            def emit_jt(jt):
                e_sb = attnp.tile([128, 16 * SS], bf16, tag="e", bufs=3)
                for g in range(4):
                    sc_ps = psB.tile([128, 4 * SS], f32, tag="sc", bufs=2)
                    for u in range(2):
                        for par in range(2):
                            h = 4 * g + 2 * u + par
                            t = h // 2
                            sc_last[jt] = nc.tensor.matmul(
                                sc_ps[:, (u if par == 0 else 2 + u) * SS :][:, :SS],
                                KhT_sb[64 * par : 64 * par + 64,
                                       t * SEQ + jt * 128 : t * SEQ + (jt + 1) * 128],
                                QhT_sb[64 * par : 64 * par + 64,
                                       t * SS : (t + 1) * SS],
                                start=True, stop=True,
                            )
                    nc.scalar.activation(
                        e_sb[:, g * 4 * SS : (g + 1) * 4 * SS], sc_ps[:],
                        AF.Exp, bias=zb[:],
                    )
                    # Pool tree (heads 0-7) starts right after exp group 1
                    if g == 1:
                        a1 = attnp.tile([128, 4 * SS], bf16, tag="a1", bufs=2)
                        nc.gpsimd.tensor_add(a1[:], e_sb[:, 0 : 4 * SS],
                                             e_sb[:, 4 * SS : 8 * SS])
                        a2 = attnp.tile([128, 2 * SS], bf16, tag="a2", bufs=2)
                        nc.gpsimd.tensor_add(a2[:], a1[:, 0 : 2 * SS],
                                             a1[:, 2 * SS : 4 * SS])
                        a3 = attnp.tile([128, SS], bf16, tag="a3", bufs=2)
                        nc.gpsimd.tensor_add(a3[:], a2[:, 0:SS], a2[:, SS : 2 * SS])
                # DVE tree (heads 8-15), combine, recip, normalize (2 halves)
                b1 = attnp.tile([128, 4 * SS], bf16, tag="b1", bufs=2)
                nc.vector.tensor_add(b1[:], e_sb[:, 8 * SS : 12 * SS],
                                     e_sb[:, 12 * SS : 16 * SS])
                b2 = attnp.tile([128, 2 * SS], bf16, tag="b2", bufs=2)
                nc.vector.tensor_add(b2[:], b1[:, 0 : 2 * SS], b1[:, 2 * SS : 4 * SS])
                b3 = attnp.tile([128, SS], bf16, tag="b3", bufs=2)
                nc.vector.tensor_add(b3[:], b2[:, 0:SS], b2[:, SS : 2 * SS])
                Dsum = attnp.tile([128, SS], f32, tag="Dsum", bufs=2)
                nc.vector.tensor_add(Dsum[:], a3[:], b3[:])
                Rf = attnp.tile([128, SS], f32, tag="Rf", bufs=2)
                nc.vector.reciprocal_approx_fast(Rf[:], Dsum[:])
                Rcp = attnp.tile([128, SS], bf16, tag="Rcp", bufs=2)
                nc.vector.tensor_copy(Rcp[:], Rf[:])
                attn = attnp.tile([128, 16 * SS], bf16, tag="attn", bufs=3)
                for half in range(2):
                    nc.vector.tensor_mul(
                        attn[:, half * 8 * SS : (half + 1) * 8 * SS].rearrange(
                            "p (s j) -> p s j", s=8),
                        e_sb[:, half * 8 * SS : (half + 1) * 8 * SS].rearrange(
                            "p (s j) -> p s j", s=8),
                        Rcp[:].unsqueeze(1).broadcast_to([128, 8, SS]),
                    )
                attn_q.append((jt, attn))
                if len(attn_q) > 2:
                    j0, a0 = attn_q.pop(0)
                    emit_ctx_half(j0, a0, 0)
                    emit_ctx_half(j0, a0, 1)

            emit_jt(0)

            # ctx accumulator: zero the 4 banks via one full-bank
            # start=True matmul each (runs in early-loop PE slack).
            ctx_ps = psB.tile([128, 2 * DIM], f32, tag="ctx", name="ctx_ps")
            for b in range(4):
                nc.tensor.matmul(
                    ctx_ps[:, 512 * b : 512 * (b + 1)],
                    z512[:, 0:128], z512[:, 0:512],
                    start=True, stop=False, skip_group_check=True,
                )

            for jt in range(1, 16):
                emit_jt(jt)
            while attn_q:
                j0, a0 = attn_q.pop(0)
                emit_ctx_half(j0, a0, 0)
                emit_ctx_half(j0, a0, 1)

            # keep the PE p-state warm through the softmax-flush idle so
            # the output projection charges full-speed cycles
            warm2 = psB.tile([128, 4 * SS], f32, tag="sc", bufs=2, name="warm2")
            for _ in range(120):
                nc.tensor.matmul(warm2[0:64, 0:64], ident[:, 0:64],
                                 ident[:, 0:64], start=True, stop=True)

            # ---------------- Phase C: output projection --------------------
            with tc.tile_pool(name="cpool", bufs=1) as cpool:
                ctx_sb = cpool.tile([128, 2 * DIM], bf16)
                ctxT_sb = cpool.tile([128, 2 * DIM], bf16)
                # per-bank ctx copies (split ACT/DVE), each chased by a
                # 512-wide xbar DMA transpose into the d-tile layout
                # ctxT[(dt) d, it*128 + i]. No PE or PSUM involved.
                ctxT_v = ctxT_sb[:].rearrange(
                    "p (dt itt i) -> p dt itt i", dt=8, itt=2)
                for idx, (it, bh) in enumerate(
                        ((0, 0), (1, 0), (0, 1), (1, 1))):
                    b = it * 2 + bh
                    src = ctx_ps[:, b * 512 : (b + 1) * 512]
                    dst = ctx_sb[:, b * 512 : (b + 1) * 512]
                    if idx % 2 == 0:
                        nc.scalar.activation(dst, src, AF.Copy)
                    else:
                        nc.vector.tensor_copy(dst, src)
                    nc.sync.dma_start_transpose(
                        ctxT_v[:, 4 * bh : 4 * bh + 4, it, :], dst)
                # O = ctx @ Wo^T + bo : out[(it) i, n], N=512 per matmul;
                # output copied (ACT) and stored in 512-wide chunks on
                # alternating DMA rings (SP / ACT).
                out_sb = cpool.tile([128, 2 * DIM], f32)
                # the 4 output-projection groups alternate between the two
                # sc-tag tiles so a group's start never waits on the
                # previous group's PSUM reader in the same tile
                ops_tiles = [
                    psB.tile([128, 4 * SS], f32, tag="sc", bufs=2,
                             name=f"ops{i}")
                    for i in range(2)
                ]
                for it in range(2):
                    for nh in range(2):
                        g = 2 * it + nh
                        ops = ops_tiles[g % 2][:, (g // 2) * 512 :][:, 0:512]
                        for kt in range(8):
                            nc.tensor.matmul(
                                ops,
                                ctxT_sb[:, kt * SS + it * 128 : kt * SS + (it + 1) * 128],
                                WoT_sb[:, kt * DIM + nh * 512 : kt * DIM + (nh + 1) * 512],
                                start=(kt == 0), stop=False,
                                skip_group_check=True,
                            )
                        nc.tensor.matmul(
                            ops, ones[:, 0:128],
                            bo_sb[:, nh * 512 : (nh + 1) * 512],
                            start=False, stop=True,
                            skip_group_check=True,
                        )
                        osl = slice(it * DIM + nh * 512, it * DIM + (nh + 1) * 512)
                        nc.scalar.activation(out_sb[:, osl], ops, AF.Copy)
                        dma_eng = nc.sync if g % 2 == 0 else nc.scalar
                        dma_eng.dma_start(
                            out.ap().rearrange(
                                "(mt p) (nh d) -> p mt nh d", p=128, nh=2
                            )[:, it, nh, :],
                            out_sb[:, osl],
                        )


def get_nc():
    if "nc" not in _CACHE:
        _CACHE["nc"] = _build()
    return _CACHE["nc"]


def make_in_maps(inputs):
    f = lambda x: np.ascontiguousarray(np.asarray(x, dtype=np.float32))
    bf = ml_dtypes.bfloat16
    q, k, v = f(inputs["q"]), f(inputs["k"]), f(inputs["v"])
    WqTs = np.ascontiguousarray((f(inputs["Wq"]) * SCALE).T.astype(bf))
    WkT = np.ascontiguousarray(f(inputs["Wk"]).T.astype(bf))
    WvT = np.ascontiguousarray(f(inputs["Wv"]).T.astype(bf))
    WoT = np.ascontiguousarray(f(inputs["Wo"]).T.astype(bf))
    bqs = f(inputs["bq"]) * np.float32(SCALE)
    bk, bv, bo = f(inputs["bk"]), f(inputs["bv"]), f(inputs["bo"])
    in_maps = []
    for c in range(NCORES):
        sl = slice(c * SS, (c + 1) * SS)
        in_maps.append({
            "qT": np.ascontiguousarray(q[sl].T.astype(bf)),
            "kT": np.ascontiguousarray(k[sl].T.astype(bf)),
            "vT": np.ascontiguousarray(v[sl].T.astype(bf)),
            "WqT": WqTs, "WkT": WkT, "WvT": WvT, "WoT": WoT,
            "bq": bqs, "bk": bk, "bv": bv, "bo": bo,
        })
    return in_maps


def run(inputs, **kwargs):
    """Run on hardware; returns (output, BassKernelResults)."""
    from concourse import bass_utils

    nc = get_nc()
    res = bass_utils.run_bass_kernel_spmd(
        nc, make_in_maps(inputs), core_ids=list(range(NCORES)), **kwargs
    )
    rows = [res.results[c]["out"] for c in range(NCORES)]
    full = np.concatenate(rows, axis=0).astype(np.float32)
    return full.reshape(1, SEQ, DIM), res


def kernel(**inputs) -> np.ndarray:
    out, _ = run(inputs)
    return out


# revision 38
# speedup vs baseline: 1.0078x; 1.0013x over previous
"""Trainium2 Bass kernel for nn_MultiHeadAttention_77446850281793.

Reference semantics (faithful quirk: softmax over the HEADS axis):
    Qh = q @ Wq.T + bq   (per-head view)   [S, H, dk]
    scores[h, i, j] = (Qh[i,h] . Kh[j,h]) / sqrt(dk)
    attn = softmax over h (heads) of scores
    ctx[h, i] = sum_j attn[h,i,j] * Vh[j,h]
    out = concat(ctx) @ Wo.T + bo

Sharding: sequence-parallel over the 8 cores (256 query rows each).
Each core projects its own 256-row slice of q/k/v; K^T and V slices are
AllGathered (bf16) so every core holds full K/V; the head-axis softmax is
then entirely core-local. Output rows are gathered on the host.

Schedule notes (cost-model driven):
  - All matmul operands bf16: 1 cycle/row everywhere.
  - Phase A runs K -> V -> Q projections, kt-outer, paced by chunked
    weight DMAs (warm-up matmuls burn the PE p-state ramp first). The
    K and V AllGathers and all 14 remote-block readbacks are in flight
    before the attention loop starts; the readbacks interleave K/V on
    the SP ring in deadline order, with the Wo load last.
  - Attention loop per j-tile: 16 score matmuls (4-head PSUM groups,
    double buffered); exp on ACT (the pacer); head-sum tree split Pool
    (heads 0-7, starting right after exp group 1) / DVE (heads 8-15);
    normalization in 2 half-muls so the ctx matmuls can chase.
  - ctx accumulates in the swapped layout [(i-tile) i, (h,dk)]: full
    128-partition outputs, N=64 per matmul -> half the PE rows.
  - Output projection: per-bank ctx copies (ACT/DVE), 16 PE transposes
    rotated over 8 PSUM slots, N=512 matmuls against bf16 WoT, output
    copied/stored in 512-wide chunks.
"""

import numpy as np
import ml_dtypes

SEQ, DIM, HEADS, DK, NCORES = 2048, 1024, 16, 64, 8
SS = SEQ // NCORES  # 256 query rows per core
SCALE = 1.0 / 8.0  # 1/sqrt(DK); folded into Wq/bq on the host

_CACHE = {}


def _build(fake_ag=False):
    import concourse.bass as bass
    import concourse.bacc as bacc
    import concourse.tile as tile
    import concourse.mybir as mybir

    dt = mybir.dt
    f32, bf16 = dt.float32, dt.bfloat16

    nc = bacc.Bacc(
        "TRN2", target_bir_lowering=False, debug=False, num_devices=NCORES
    )

    qT = nc.dram_tensor("qT", [DIM, SS], bf16, kind="ExternalInput")
    kT = nc.dram_tensor("kT", [DIM, SS], bf16, kind="ExternalInput")
    vT = nc.dram_tensor("vT", [DIM, SS], bf16, kind="ExternalInput")
    WqT = nc.dram_tensor("WqT", [DIM, DIM], bf16, kind="ExternalInput")
    WkT = nc.dram_tensor("WkT", [DIM, DIM], bf16, kind="ExternalInput")
    WvT = nc.dram_tensor("WvT", [DIM, DIM], bf16, kind="ExternalInput")
    WoT = nc.dram_tensor("WoT", [DIM, DIM], bf16, kind="ExternalInput")
    bq = nc.dram_tensor("bq", [DIM], f32, kind="ExternalInput")
    bk = nc.dram_tensor("bk", [DIM], f32, kind="ExternalInput")
    bv = nc.dram_tensor("bv", [DIM], f32, kind="ExternalInput")
    bo = nc.dram_tensor("bo", [DIM], f32, kind="ExternalInput")
    out = nc.dram_tensor("out", [SS, DIM], f32, kind="ExternalOutput")

    with tile.TileContext(nc) as tc:
        _emit(nc, tc, bass, mybir, locals(), fake_ag=fake_ag)
    nc.compile()
    return nc


def _emit(nc, tc, bass, mybir, io, fake_ag=False):
    dt = mybir.dt
    f32, bf16 = dt.float32, dt.bfloat16
    AF = mybir.ActivationFunctionType
    qT, kT, vT = io["qT"], io["kT"], io["vT"]
    WqT, WkT, WvT, WoT = io["WqT"], io["WkT"], io["WvT"], io["WoT"]
    bq, bk, bv, bo = io["bq"], io["bk"], io["bv"], io["bo"]
    out = io["out"]

    # head h -> column slot in the per-j-tile score/exp buffers (the two
    # heads of a row-packed matmul pair go to different PSUM banks).
    def slot_col(h):
        g, u, par = h // 4, (h % 4) // 2, h % 2
        slot = u if par == 0 else 2 + u
        return g * 4 * SS + slot * SS

    with (
        tc.tile_pool(name="constp", bufs=1) as constp,
        tc.tile_pool(name="qhtp", bufs=1) as qhtp,
        tc.tile_pool(name="kvp", bufs=1) as kvp,
        tc.tile_pool(name="dramp", bufs=1, space="DRAM") as dramp,
    ):
        ones = constp.tile([1, 128], bf16)
        nc.gpsimd.memset(ones[:], 1.0)
        zb = constp.tile([128, 1], f32)
        nc.gpsimd.memset(zb[:], 0.0)
        z512 = constp.tile([1, 512], bf16)
        nc.gpsimd.memset(z512[:], 0.0)
        ident = constp.tile([128, 128], bf16)
        from concourse.masks import make_identity
        make_identity(nc, ident[:])

        bq_sb = constp.tile([128, 8], f32)
        bk_sb = constp.tile([128, 8], f32)
        bv_sb = constp.tile([1, DIM], bf16)
        bo_sb = constp.tile([1, DIM], bf16)

        aspace = "Local" if fake_ag else "Shared"
        ag_in_k = dramp.tile([DIM, SS], bf16)
        ag_in_v = dramp.tile([DIM, SS], bf16)
        ag_out_k = dramp.tile([NCORES * DIM, SS], bf16, addr_space=aspace)
        ag_out_v = dramp.tile([NCORES * DIM, SS], bf16, addr_space=aspace)

        QhT_sb = qhtp.tile([128, 8 * SS], bf16)
        KhT_c2 = qhtp.tile([128, 8 * SS], bf16)

        # long-lived attention operands
        KhT_sb = kvp.tile([128, 8 * SEQ], bf16)
        Vh_sb = kvp.tile([128, 16 * DIM], bf16)
        WoT_sb = kvp.tile([128, 8 * DIM], bf16)
        KhT_v = KhT_sb[:].rearrange("p (t j) -> p t j", t=8)
        Vh_v = Vh_sb[:].rearrange("p (jt d) -> p jt d", jt=16)

        pid = nc.partition_id()

        def load_w(pool, dram_w, name, nchunks=8):
            w_sb = pool.tile([128, 8 * DIM], bf16, name=name)
            src = dram_w.ap().rearrange("(t p) d -> p t d", p=128)
            dst = w_sb[:].rearrange("p (t d) -> p t d", t=8)
            step = 8 // nchunks
            for h in range(nchunks):
                nc.sync.dma_start(
                    dst[:, step * h : step * (h + 1), :],
                    src[:, step * h : step * (h + 1), :],
                )
            return w_sb

        def load_x(pool, dram_x, name):
            x_sb = pool.tile([128, 8 * SS], bf16, name=name)
            nc.sync.dma_start(
                x_sb[:].rearrange("p (t j) -> p t j", t=8),
                dram_x.ap().rearrange("(t p) j -> p t j", p=128),
            )
            return x_sb

        # kt-outer projection: 8 half-bank accumulators in 4 PSUM tiles;
        # matmuls for chunk kt start as soon as that chunk of W lands and
        # the PSUM->SBUF bias-copies (alternating DVE/ACT) chase the last
        # chunk's per-mt stops.
        def project_T(psA, x_sb, w_sb, bias_sb, dst_sb, tag):
            # one accumulation group per PSUM bank: start=True clears the
            # whole 2KB bank on HW, so mt-groups may NOT share banks
            ps = [
                psA.tile([128, 512], f32, tag=f"{tag}{m}", name=f"ps_{tag}{m}")
                for m in range(8)
            ]
            for kt in range(8):
                for mt in range(8):
                    nc.tensor.matmul(
                        ps[mt][:, 0:SS],
                        w_sb[:, kt * DIM + mt * 128 : kt * DIM + (mt + 1) * 128],
                        x_sb[:, kt * SS : (kt + 1) * SS],
                        start=(kt == 0), stop=(kt == 7),
                    )
            for mt in range(8):
                src = ps[mt][:, 0:SS]
                dst = dst_sb[:, mt * SS : (mt + 1) * SS]
                if mt % 2 == 0:
                    nc.vector.tensor_scalar_add(dst, src, bias_sb[:, mt : mt + 1])
                else:
                    nc.scalar.activation(dst, src, AF.Identity,
                                         bias=bias_sb[:, mt : mt + 1], scale=1.0)

        # ---------------- Phase A: K, V, Q projections ----------------------
        with (
            tc.tile_pool(name="wpk", bufs=1) as wpk,
            tc.tile_pool(name="wpvq", bufs=1) as wpvq,
            tc.tile_pool(name="psA", bufs=1, space="PSUM") as psA,
        ):
            kT_sb = load_x(wpk, kT, "kT_sb")
            WkT_sb = load_w(wpk, WkT, "WkT_sb")
            nc.sync.dma_start(bk_sb[:], bk.ap().rearrange("(t p) -> p t", p=128))
            nc.sync.dma_start(bq_sb[:], bq.ap().rearrange("(t p) -> p t", p=128))
            # casting (f32 -> bf16) bias DMAs ride the SWDGE ring
            nc.gpsimd.dma_start(bv_sb[:], bv.ap().unsqueeze(0))
            nc.gpsimd.dma_start(bo_sb[:], bo.ap().unsqueeze(0))
            vT_sb = load_x(wpvq, vT, "vT_sb")
            WvT_sb = load_w(wpvq, WvT, "WvT_sb")
            qT_sb = load_x(wpvq, qT, "qT_sb")
            WqT_sb = load_w(wpvq, WqT, "WqT_sb")

            # p-state warm-up while the first weight chunks stream in
            # (borrows the first projection bank; WAW keeps it ordered)
            warm = psA.tile([128, 512], f32, tag="kq0", name="warm")
            for _ in range(24):
                nc.tensor.matmul(warm[0:64, 0:64], ident[:, 0:64], ident[:, 0:64],
                                 start=True, stop=True)

            # K projection + AllGather staging (SWDGE ring)
            project_T(psA, kT_sb, WkT_sb, bk_sb, KhT_c2, "kq")
            nc.gpsimd.dma_start(
                ag_in_k[:, :].rearrange("(t p) j -> p t j", p=128),
                KhT_c2[:].rearrange("p (t j) -> p t j", t=8),
            )
            # own K block: SBUF->SBUF on DVE, early (DVE is idle here)
            nc.vector.tensor_copy(KhT_v[:, :, 0:SS],
                                  KhT_c2[:].rearrange("p (t j) -> p t j", t=8))
            if fake_ag:
                nc.gpsimd.dma_start(
                    ag_out_k[:, :].rearrange("(c r) j -> c r j", c=NCORES)[0],
                    ag_in_k[:, :])
            else:
                nc.gpsimd.collective_compute(
                    "AllGather", mybir.AluOpType.bypass,
                    replica_groups=[list(range(NCORES))],
                    ins=[ag_in_k[:, :]], outs=[ag_out_k[:, :]],
                )

            # V projection straight into the own-block slots of Vh_sb
            # (copies on the otherwise idle ACT)
            for st in range(2):
                for nh in range(2):
                    vps = psA.tile([128, 512], f32, tag=f"kq{2*st+nh}",
                                   name="vps")
                    for kt in range(8):
                        nc.tensor.matmul(
                            vps[:],
                            vT_sb[:, kt * SS + st * 128 : kt * SS + (st + 1) * 128],
                            WvT_sb[:, kt * DIM + nh * 512 : kt * DIM + (nh + 1) * 512],
                            start=(kt == 0), stop=False,
                        )
                    nc.tensor.matmul(
                        vps[:], ones[:, 0:128],
                        bv_sb[:, nh * 512 : (nh + 1) * 512],
                        start=False, stop=True,
                    )
                    nc.scalar.activation(
                        Vh_sb[:, st * DIM + nh * 512 : st * DIM + (nh + 1) * 512],
                        vps[:], AF.Copy,
                    )

            # Q projection (scale pre-folded into WqT/bq on host)
            project_T(psA, qT_sb, WqT_sb, bq_sb, QhT_sb, "kq")

            # V AllGather chain + all readbacks, interleaved in deadline
            # order on the SP ring (ring order = emission order); by the
            # time SP reaches ag_in_v its inputs are long done.
            nc.sync.dma_start(
                ag_in_v[:, :].rearrange("(a p c) j -> p a (c j)", a=2, p=128),
                Vh_sb[:, 0 : 2 * DIM].rearrange("p (a d) -> p a d", a=2),
            )
            if fake_ag:
                nc.sync.dma_start(
                    ag_out_v[:, :].rearrange("(c r) j -> c r j", c=NCORES)[0],
                    ag_in_v[:, :])
            else:
                nc.gpsimd.collective_compute(
                    "AllGather", mybir.AluOpType.bypass,
                    replica_groups=[list(range(NCORES))],
                    ins=[ag_in_v[:, :]], outs=[ag_out_v[:, :]],
                )

            def emit_rbk(s):
                blk = (pid + s) % NCORES
                nc.sync.dma_start(
                    KhT_v[:, :, SS * s : SS * (s + 1)],
                    ag_out_k[bass.ds(blk * DIM, DIM), :].rearrange(
                        "(t p) j -> p t j", p=128),
                )

            def emit_rbv(s):
                blk = (pid + s) % NCORES
                nc.sync.dma_start(
                    Vh_v[:, 2 * s : 2 * s + 2, :],
                    ag_out_v[bass.ds(blk * DIM, DIM), :].rearrange(
                        "(a p c2) j -> p a (c2 j)", a=2, p=128),
                )

            emit_rbk(1)
            emit_rbk(2)
            emit_rbv(1)
            emit_rbk(3)
            emit_rbv(2)
            for s in range(4, NCORES):
                emit_rbk(s)
                emit_rbv(s - 1)
            emit_rbv(7)
            # Wo load trails everything on the SP ring
            wo_src = WoT.ap().rearrange("(t p) d -> p t d", p=128)
            wo_dst = WoT_sb[:].rearrange("p (t d) -> p t d", t=8)
            for h in range(2):
                nc.sync.dma_start(wo_dst[:, 4 * h : 4 * h + 4, :],
                                  wo_src[:, 4 * h : 4 * h + 4, :])

        # ---------------- Phase B: attention over full K/V ------------------
        with (
            tc.tile_pool(name="attnp", bufs=2) as attnp,
            tc.tile_pool(name="psB", bufs=1, space="PSUM") as psB,
        ):
            attn_q = []
            sc_last = {}
            from concourse.tile import add_dep_helper

            ctx_ps = None  # allocated after j-tile 0's emission

            def emit_ctx_half(jt, attn, half):
                for h in range(8 * half, 8 * half + 8):
                    for it in range(2):
                        is_stop = jt == 15 and (h % 8) == 7
                        mm = nc.tensor.matmul(
                            ctx_ps[:, it * DIM + h * DK : it * DIM + (h + 1) * DK],
                            attn[:, slot_col(h) + it * 128 : slot_col(h) + (it + 1) * 128],
                            Vh_sb[:, jt * DIM + h * DK : jt * DIM + (h + 1) * DK],
                            start=False, stop=is_stop,
                            skip_group_check=True,
                        )
                        if h % 8 == 0 and it == 0 and (jt + 1) in sc_last:
                            add_dep_helper(
                                mm.ins, sc_last[jt + 1].ins, sync=False,
                                reason="scores ahead of ctx on PE",
                            )

            def emit_jt(jt):
                e_sb = attnp.tile([128, 16 * SS], bf16, tag="e", bufs=3)
                for g in range(4):
                    sc_ps = psB.tile([128, 4 * SS], f32, tag="sc", bufs=2)
                    for u in range(2):
                        for par in range(2):
                            h = 4 * g + 2 * u + par
                            t = h // 2
                            sc_last[jt] = nc.tensor.matmul(
                                sc_ps[:, (u if par == 0 else 2 + u) * SS :][:, :SS],
                                KhT_sb[64 * par : 64 * par + 64,
                                       t * SEQ + jt * 128 : t * SEQ + (jt + 1) * 128],
                                QhT_sb[64 * par : 64 * par + 64,
                                       t * SS : (t + 1) * SS],
                                start=True, stop=True,
                            )
                    nc.scalar.activation(
                        e_sb[:, g * 4 * SS : (g + 1) * 4 * SS], sc_ps[:],
                        AF.Exp, bias=zb[:],
                    )
                    # Pool tree (heads 0-7) starts right after exp group 1
                    if g == 1:
                        a1 = attnp.tile([128, 4 * SS], bf16, tag="a1", bufs=2)
                        nc.gpsimd.tensor_add(a1[:], e_sb[:, 0 : 4 * SS],
                                             e_sb[:, 4 * SS : 8 * SS])
                        a2 = attnp.tile([128, 2 * SS], bf16, tag="a2", bufs=2)
                        nc.gpsimd.tensor_add(a2[:], a1[:, 0 : 2 * SS],
                                             a1[:, 2 * SS : 4 * SS])
                        a3 = attnp.tile([128, SS], bf16, tag="a3", bufs=2)
                        nc.gpsimd.tensor_add(a3[:], a2[:, 0:SS], a2[:, SS : 2 * SS])
                # DVE tree (heads 8-15), combine, recip, normalize (2 halves)
                b1 = attnp.tile([128, 4 * SS], bf16, tag="b1", bufs=2)
                nc.vector.tensor_add(b1[:], e_sb[:, 8 * SS : 12 * SS],
                                     e_sb[:, 12 * SS : 16 * SS])
                b2 = attnp.tile([128, 2 * SS], bf16, tag="b2", bufs=2)
                nc.vector.tensor_add(b2[:], b1[:, 0 : 2 * SS], b1[:, 2 * SS : 4 * SS])
                b3 = attnp.tile([128, SS], bf16, tag="b3", bufs=2)
                nc.vector.tensor_add(b3[:], b2[:, 0:SS], b2[:, SS : 2 * SS])
                Dsum = attnp.tile([128, SS], bf16, tag="Dsum", bufs=2)
                nc.vector.tensor_add(Dsum[:], a3[:], b3[:])
                Rf = attnp.tile([128, SS], f32, tag="Rf", bufs=2)
                nc.vector.reciprocal(Rf[:], Dsum[:])
                Rcp = attnp.tile([128, SS], bf16, tag="Rcp", bufs=2)
                nc.vector.tensor_copy(Rcp[:], Rf[:])
                attn = attnp.tile([128, 16 * SS], bf16, tag="attn", bufs=3)
                for half in range(2):
                    nc.vector.tensor_mul(
                        attn[:, half * 8 * SS : (half + 1) * 8 * SS].rearrange(
                            "p (s j) -> p s j", s=8),
                        e_sb[:, half * 8 * SS : (half + 1) * 8 * SS].rearrange(
                            "p (s j) -> p s j", s=8),
                        Rcp[:].unsqueeze(1).broadcast_to([128, 8, SS]),
                    )
                attn_q.append((jt, attn))
                if len(attn_q) > 2:
                    j0, a0 = attn_q.pop(0)
                    emit_ctx_half(j0, a0, 0)
                    emit_ctx_half(j0, a0, 1)

            emit_jt(0)

            # ctx accumulator: zero the 4 banks via one full-bank
            # start=True matmul each (runs in early-loop PE slack).
            ctx_ps = psB.tile([128, 2 * DIM], f32, tag="ctx", name="ctx_ps")
            for b in range(4):
                nc.tensor.matmul(
                    ctx_ps[:, 512 * b : 512 * (b + 1)],
                    z512[:, 0:128], z512[:, 0:512],
                    start=True, stop=False, skip_group_check=True,
                )

            for jt in range(1, 16):
                emit_jt(jt)
            while attn_q:
                j0, a0 = attn_q.pop(0)
                emit_ctx_half(j0, a0, 0)
                emit_ctx_half(j0, a0, 1)

            # keep the PE p-state warm through the softmax-flush idle so
            # the output projection charges full-speed cycles
            warm2 = psB.tile([128, 4 * SS], f32, tag="sc", bufs=2, name="warm2")
            for _ in range(120):
                nc.tensor.matmul(warm2[0:64, 0:64], ident[:, 0:64],
                                 ident[:, 0:64], start=True, stop=True)

            # ---------------- Phase C: output projection --------------------
            with tc.tile_pool(name="cpool", bufs=1) as cpool:
                ctx_sb = cpool.tile([128, 2 * DIM], bf16)
                ctxT_sb = cpool.tile([128, 2 * DIM], bf16)
                # per-bank ctx copies (split ACT/DVE), each chased by a
                # 512-wide xbar DMA transpose into the d-tile layout
                # ctxT[(dt) d, it*128 + i]. No PE or PSUM involved.
                ctxT_v = ctxT_sb[:].rearrange(
                    "p (dt itt i) -> p dt itt i", dt=8, itt=2)
                for idx, (it, bh) in enumerate(
                        ((0, 0), (1, 0), (0, 1), (1, 1))):
                    b = it * 2 + bh
                    src = ctx_ps[:, b * 512 : (b + 1) * 512]
                    dst = ctx_sb[:, b * 512 : (b + 1) * 512]
                    if idx % 2 == 0:
                        nc.scalar.activation(dst, src, AF.Copy)
                    else:
                        nc.vector.tensor_copy(dst, src)
                    nc.sync.dma_start_transpose(
                        ctxT_v[:, 4 * bh : 4 * bh + 4, it, :], dst)
                # O = ctx @ Wo^T + bo : out[(it) i, n], N=512 per matmul;
                # output copied (ACT) and stored in 512-wide chunks on
                # alternating DMA rings (SP / ACT).
                out_sb = cpool.tile([128, 2 * DIM], f32)
                # the 4 output-projection groups alternate between the two
                # sc-tag tiles so a group's start never waits on the
                # previous group's PSUM reader in the same tile
                ops_tiles = [
                    psB.tile([128, 4 * SS], f32, tag="sc", bufs=2,
                             name=f"ops{i}")
                    for i in range(2)
                ]
                for it in range(2):
                    for nh in range(2):
                        g = 2 * it + nh
                        ops = ops_tiles[g % 2][:, (g // 2) * 512 :][:, 0:512]
                        for kt in range(8):
                            nc.tensor.matmul(
                                ops,
                                ctxT_sb[:, kt * SS + it * 128 : kt * SS + (it + 1) * 128],
                                WoT_sb[:, kt * DIM + nh * 512 : kt * DIM + (nh + 1) * 512],
                                start=(kt == 0), stop=False,
                                skip_group_check=True,
                            )
                        nc.tensor.matmul(
                            ops, ones[:, 0:128],
                            bo_sb[:, nh * 512 : (nh + 1) * 512],
                            start=False, stop=True,
                            skip_group_check=True,
                        )
                        osl = slice(it * DIM + nh * 512, it * DIM + (nh + 1) * 512)
                        nc.scalar.activation(out_sb[:, osl], ops, AF.Copy)
                        dma_eng = nc.sync if g % 2 == 0 else nc.scalar
                        dma_eng.dma_start(
                            out.ap().rearrange(
                                "(mt p) (nh d) -> p mt nh d", p=128, nh=2
                            )[:, it, nh, :],
                            out_sb[:, osl],
                        )


def get_nc():
    if "nc" not in _CACHE:
        _CACHE["nc"] = _build()
    return _CACHE["nc"]


def make_in_maps(inputs):
    f = lambda x: np.ascontiguousarray(np.asarray(x, dtype=np.float32))
    bf = ml_dtypes.bfloat16
    q, k, v = f(inputs["q"]), f(inputs["k"]), f(inputs["v"])
    WqTs = np.ascontiguousarray((f(inputs["Wq"]) * SCALE).T.astype(bf))
    WkT = np.ascontiguousarray(f(inputs["Wk"]).T.astype(bf))
    WvT = np.ascontiguousarray(f(inputs["Wv"]).T.astype(bf))
    WoT = np.ascontiguousarray(f(inputs["Wo"]).T.astype(bf))
    bqs = f(inputs["bq"]) * np.float32(SCALE)
    bk, bv, bo = f(inputs["bk"]), f(inputs["bv"]), f(inputs["bo"])
    in_maps = []
    for c in range(NCORES):
        sl = slice(c * SS, (c + 1) * SS)
        in_maps.append({
            "qT": np.ascontiguousarray(q[sl].T.astype(bf)),
            "kT": np.ascontiguousarray(k[sl].T.astype(bf)),
            "vT": np.ascontiguousarray(v[sl].T.astype(bf)),
            "WqT": WqTs, "WkT": WkT, "WvT": WvT, "WoT": WoT,
            "bq": bqs, "bk": bk, "bv": bv, "bo": bo,
        })
    return in_maps


def run(inputs, **kwargs):
    """Run on hardware; returns (output, BassKernelResults)."""
    from concourse import bass_utils

    nc = get_nc()
    res = bass_utils.run_bass_kernel_spmd(
        nc, make_in_maps(inputs), core_ids=list(range(NCORES)), **kwargs
    )
    rows = [res.results[c]["out"] for c in range(NCORES)]
    full = np.concatenate(rows, axis=0).astype(np.float32)
    return full.reshape(1, SEQ, DIM), res


def kernel(**inputs) -> np.ndarray:
    out, _ = run(inputs)
    return out


# revision 41
# speedup vs baseline: 1.0102x; 1.0024x over previous
"""Trainium2 Bass kernel for nn_MultiHeadAttention_77446850281793.

Reference semantics (faithful quirk: softmax over the HEADS axis):
    Qh = q @ Wq.T + bq   (per-head view)   [S, H, dk]
    scores[h, i, j] = (Qh[i,h] . Kh[j,h]) / sqrt(dk)
    attn = softmax over h (heads) of scores
    ctx[h, i] = sum_j attn[h,i,j] * Vh[j,h]
    out = concat(ctx) @ Wo.T + bo

Sharding: sequence-parallel over the 8 cores (256 query rows each).
Each core projects its own 256-row slice of q/k/v; K^T and V slices are
AllGathered (bf16) so every core holds full K/V; the head-axis softmax is
then entirely core-local. Output rows are gathered on the host.

Schedule notes (cost-model driven):
  - All matmul operands bf16: 1 cycle/row everywhere.
  - Phase A runs K -> V -> Q projections, kt-outer, paced by chunked
    weight DMAs (warm-up matmuls burn the PE p-state ramp first). The
    K and V AllGathers and all 14 remote-block readbacks are in flight
    before the attention loop starts; the readbacks interleave K/V on
    the SP ring in deadline order, with the Wo load last.
  - Attention loop per j-tile: 16 score matmuls (4-head PSUM groups,
    double buffered); exp on ACT (the pacer); head-sum tree split Pool
    (heads 0-7, starting right after exp group 1) / DVE (heads 8-15);
    normalization in 2 half-muls so the ctx matmuls can chase.
  - ctx accumulates in the swapped layout [(i-tile) i, (h,dk)]: full
    128-partition outputs, N=64 per matmul -> half the PE rows.
  - Output projection: per-bank ctx copies (ACT/DVE), 16 PE transposes
    rotated over 8 PSUM slots, N=512 matmuls against bf16 WoT, output
    copied/stored in 512-wide chunks.
"""

import numpy as np
import ml_dtypes

SEQ, DIM, HEADS, DK, NCORES = 2048, 1024, 16, 64, 8
SS = SEQ // NCORES  # 256 query rows per core
SCALE = 1.0 / 8.0  # 1/sqrt(DK); folded into Wq/bq on the host

_CACHE = {}


def _build(fake_ag=False):
    import concourse.bass as bass
    import concourse.bacc as bacc
    import concourse.tile as tile
    import concourse.mybir as mybir

    dt = mybir.dt
    f32, bf16 = dt.float32, dt.bfloat16

    nc = bacc.Bacc(
        "TRN2", target_bir_lowering=False, debug=False, num_devices=NCORES
    )

    qT = nc.dram_tensor("qT", [DIM, SS], bf16, kind="ExternalInput")
    kT = nc.dram_tensor("kT", [DIM, SS], bf16, kind="ExternalInput")
    vT = nc.dram_tensor("vT", [DIM, SS], bf16, kind="ExternalInput")
    WqT = nc.dram_tensor("WqT", [DIM, DIM], bf16, kind="ExternalInput")
    WkT = nc.dram_tensor("WkT", [DIM, DIM], bf16, kind="ExternalInput")
    WvT = nc.dram_tensor("WvT", [DIM, DIM], bf16, kind="ExternalInput")
    WoT = nc.dram_tensor("WoT", [DIM, DIM], bf16, kind="ExternalInput")
    bq = nc.dram_tensor("bq", [DIM], f32, kind="ExternalInput")
    bk = nc.dram_tensor("bk", [DIM], f32, kind="ExternalInput")
    bv = nc.dram_tensor("bv", [DIM], f32, kind="ExternalInput")
    bo = nc.dram_tensor("bo", [DIM], f32, kind="ExternalInput")
    out = nc.dram_tensor("out", [SS, DIM], f32, kind="ExternalOutput")

    with tile.TileContext(nc) as tc:
        _emit(nc, tc, bass, mybir, locals(), fake_ag=fake_ag)
    nc.compile()
    return nc


def _emit(nc, tc, bass, mybir, io, fake_ag=False):
    dt = mybir.dt
    f32, bf16 = dt.float32, dt.bfloat16
    AF = mybir.ActivationFunctionType
    qT, kT, vT = io["qT"], io["kT"], io["vT"]
    WqT, WkT, WvT, WoT = io["WqT"], io["WkT"], io["WvT"], io["WoT"]
    bq, bk, bv, bo = io["bq"], io["bk"], io["bv"], io["bo"]
    out = io["out"]

    # head h -> column slot in the per-j-tile score/exp buffers (the two
    # heads of a row-packed matmul pair go to different PSUM banks).
    def slot_col(h):
        g, u, par = h // 4, (h % 4) // 2, h % 2
        slot = u if par == 0 else 2 + u
        return g * 4 * SS + slot * SS

    with (
        tc.tile_pool(name="constp", bufs=1) as constp,
        tc.tile_pool(name="qhtp", bufs=1) as qhtp,
        tc.tile_pool(name="kvp", bufs=1) as kvp,
        tc.tile_pool(name="dramp", bufs=1, space="DRAM") as dramp,
    ):
        ones = constp.tile([1, 128], bf16)
        nc.gpsimd.memset(ones[:], 1.0)
        zb = constp.tile([128, 1], f32)
        nc.gpsimd.memset(zb[:], 0.0)
        z512 = constp.tile([1, 512], bf16)
        nc.gpsimd.memset(z512[:], 0.0)
        ident = constp.tile([128, 128], bf16)
        from concourse.masks import make_identity
        make_identity(nc, ident[:])

        bq_sb = constp.tile([128, 8], f32)
        bk_sb = constp.tile([128, 8], f32)
        bv_sb = constp.tile([1, DIM], bf16)
        bo_sb = constp.tile([1, DIM], bf16)

        aspace = "Local" if fake_ag else "Shared"
        ag_in_k = dramp.tile([DIM, SS], bf16)
        ag_in_v = dramp.tile([DIM, SS], bf16)
        ag_out_k = dramp.tile([NCORES * DIM, SS], bf16, addr_space=aspace)
        ag_out_v = dramp.tile([NCORES * DIM, SS], bf16, addr_space=aspace)

        QhT_sb = qhtp.tile([128, 8 * SS], bf16)
        KhT_c2 = qhtp.tile([128, 8 * SS], bf16)

        # long-lived attention operands
        KhT_sb = kvp.tile([128, 8 * SEQ], bf16)
        Vh_sb = kvp.tile([128, 16 * DIM], bf16)
        WoT_sb = kvp.tile([128, 8 * DIM], bf16)
        KhT_v = KhT_sb[:].rearrange("p (t j) -> p t j", t=8)
        Vh_v = Vh_sb[:].rearrange("p (jt d) -> p jt d", jt=16)

        pid = nc.partition_id()

        def load_w(pool, dram_w, name, nchunks=8):
            w_sb = pool.tile([128, 8 * DIM], bf16, name=name)
            src = dram_w.ap().rearrange("(t p) d -> p t d", p=128)
            dst = w_sb[:].rearrange("p (t d) -> p t d", t=8)
            step = 8 // nchunks
            for h in range(nchunks):
                nc.sync.dma_start(
                    dst[:, step * h : step * (h + 1), :],
                    src[:, step * h : step * (h + 1), :],
                )
            return w_sb

        def load_x(pool, dram_x, name):
            x_sb = pool.tile([128, 8 * SS], bf16, name=name)
            nc.sync.dma_start(
                x_sb[:].rearrange("p (t j) -> p t j", t=8),
                dram_x.ap().rearrange("(t p) j -> p t j", p=128),
            )
            return x_sb

        # kt-outer projection: 8 half-bank accumulators in 4 PSUM tiles;
        # matmuls for chunk kt start as soon as that chunk of W lands and
        # the PSUM->SBUF bias-copies (alternating DVE/ACT) chase the last
        # chunk's per-mt stops.
        def project_T(psA, x_sb, w_sb, bias_sb, dst_sb, tag):
            # one accumulation group per PSUM bank: start=True clears the
            # whole 2KB bank on HW, so mt-groups may NOT share banks
            ps = [
                psA.tile([128, 512], f32, tag=f"{tag}{m}", name=f"ps_{tag}{m}")
                for m in range(8)
            ]
            for kt in range(8):
                for mt in range(8):
                    nc.tensor.matmul(
                        ps[mt][:, 0:SS],
                        w_sb[:, kt * DIM + mt * 128 : kt * DIM + (mt + 1) * 128],
                        x_sb[:, kt * SS : (kt + 1) * SS],
                        start=(kt == 0), stop=(kt == 7),
                    )
            for mt in range(8):
                src = ps[mt][:, 0:SS]
                dst = dst_sb[:, mt * SS : (mt + 1) * SS]
                if mt % 2 == 0:
                    nc.vector.tensor_scalar_add(dst, src, bias_sb[:, mt : mt + 1])
                else:
                    nc.scalar.activation(dst, src, AF.Identity,
                                         bias=bias_sb[:, mt : mt + 1], scale=1.0)

        # ---------------- Phase A: K, V, Q projections ----------------------
        with (
            tc.tile_pool(name="wpk", bufs=1) as wpk,
            tc.tile_pool(name="wpvq", bufs=1) as wpvq,
            tc.tile_pool(name="psA", bufs=1, space="PSUM") as psA,
        ):
            kT_sb = load_x(wpk, kT, "kT_sb")
            WkT_sb = load_w(wpk, WkT, "WkT_sb")
            nc.sync.dma_start(bk_sb[:], bk.ap().rearrange("(t p) -> p t", p=128))
            nc.sync.dma_start(bq_sb[:], bq.ap().rearrange("(t p) -> p t", p=128))
            # casting (f32 -> bf16) bias DMAs ride the SWDGE ring
            nc.gpsimd.dma_start(bv_sb[:], bv.ap().unsqueeze(0))
            nc.gpsimd.dma_start(bo_sb[:], bo.ap().unsqueeze(0))
            vT_sb = load_x(wpvq, vT, "vT_sb")
            WvT_sb = load_w(wpvq, WvT, "WvT_sb")
            qT_sb = load_x(wpvq, qT, "qT_sb")
            WqT_sb = load_w(wpvq, WqT, "WqT_sb")

            # p-state warm-up while the first weight chunks stream in
            # (borrows the first projection bank; WAW keeps it ordered)
            warm = psA.tile([128, 512], f32, tag="kq0", name="warm")
            for _ in range(24):
                nc.tensor.matmul(warm[0:64, 0:64], ident[:, 0:64], ident[:, 0:64],
                                 start=True, stop=True)

            # K projection + AllGather staging (SWDGE ring)
            project_T(psA, kT_sb, WkT_sb, bk_sb, KhT_c2, "kq")
            nc.gpsimd.dma_start(
                ag_in_k[:, :].rearrange("(t p) j -> p t j", p=128),
                KhT_c2[:].rearrange("p (t j) -> p t j", t=8),
            )
            # own K block: SBUF->SBUF on DVE, early (DVE is idle here)
            nc.vector.tensor_copy(KhT_v[:, :, 0:SS],
                                  KhT_c2[:].rearrange("p (t j) -> p t j", t=8))
            if fake_ag:
                nc.gpsimd.dma_start(
                    ag_out_k[:, :].rearrange("(c r) j -> c r j", c=NCORES)[0],
                    ag_in_k[:, :])
            else:
                nc.gpsimd.collective_compute(
                    "AllGather", mybir.AluOpType.bypass,
                    replica_groups=[list(range(NCORES))],
                    ins=[ag_in_k[:, :]], outs=[ag_out_k[:, :]],
                )

            # V projection straight into the own-block slots of Vh_sb
            # (copies on the otherwise idle ACT)
            for st in range(2):
                for nh in range(2):
                    vps = psA.tile([128, 512], f32, tag=f"kq{2*st+nh}",
                                   name="vps")
                    for kt in range(8):
                        nc.tensor.matmul(
                            vps[:],
                            vT_sb[:, kt * SS + st * 128 : kt * SS + (st + 1) * 128],
                            WvT_sb[:, kt * DIM + nh * 512 : kt * DIM + (nh + 1) * 512],
                            start=(kt == 0), stop=False,
                        )
                    nc.tensor.matmul(
                        vps[:], ones[:, 0:128],
                        bv_sb[:, nh * 512 : (nh + 1) * 512],
                        start=False, stop=True,
                    )
                    nc.scalar.activation(
                        Vh_sb[:, st * DIM + nh * 512 : st * DIM + (nh + 1) * 512],
                        vps[:], AF.Copy,
                    )

            # Q projection (scale pre-folded into WqT/bq on host)
            project_T(psA, qT_sb, WqT_sb, bq_sb, QhT_sb, "kq")

            # V AllGather chain + all readbacks, interleaved in deadline
            # order on the SP ring (ring order = emission order); by the
            # time SP reaches ag_in_v its inputs are long done.
            nc.sync.dma_start(
                ag_in_v[:, :].rearrange("(a p c) j -> p a (c j)", a=2, p=128),
                Vh_sb[:, 0 : 2 * DIM].rearrange("p (a d) -> p a d", a=2),
            )
            if fake_ag:
                nc.sync.dma_start(
                    ag_out_v[:, :].rearrange("(c r) j -> c r j", c=NCORES)[0],
                    ag_in_v[:, :])
            else:
                nc.gpsimd.collective_compute(
                    "AllGather", mybir.AluOpType.bypass,
                    replica_groups=[list(range(NCORES))],
                    ins=[ag_in_v[:, :]], outs=[ag_out_v[:, :]],
                )

            def emit_rbk(s):
                blk = (pid + s) % NCORES
                nc.sync.dma_start(
                    KhT_v[:, :, SS * s : SS * (s + 1)],
                    ag_out_k[bass.ds(blk * DIM, DIM), :].rearrange(
                        "(t p) j -> p t j", p=128),
                )

            def emit_rbv(s):
                blk = (pid + s) % NCORES
                nc.sync.dma_start(
                    Vh_v[:, 2 * s : 2 * s + 2, :],
                    ag_out_v[bass.ds(blk * DIM, DIM), :].rearrange(
                        "(a p c2) j -> p a (c2 j)", a=2, p=128),
                )

            emit_rbk(1)
            emit_rbk(2)
            emit_rbv(1)
            emit_rbk(3)
            emit_rbv(2)
            for s in range(4, NCORES):
                emit_rbk(s)
                emit_rbv(s - 1)
            emit_rbv(7)
            # Wo load trails everything on the SP ring
            wo_src = WoT.ap().rearrange("(t p) d -> p t d", p=128)
            wo_dst = WoT_sb[:].rearrange("p (t d) -> p t d", t=8)
            for h in range(2):
                nc.sync.dma_start(wo_dst[:, 4 * h : 4 * h + 4, :],
                                  wo_src[:, 4 * h : 4 * h + 4, :])

        # ---------------- Phase B: attention over full K/V ------------------
        with (
            tc.tile_pool(name="attnp", bufs=2) as attnp,
            tc.tile_pool(name="psB", bufs=1, space="PSUM") as psB,
        ):
            attn_q = []
            sc_last = {}
            from concourse.tile import add_dep_helper

            ctx_ps = None  # allocated after j-tile 0's emission

            def emit_ctx_half(jt, attn, half):
                for h in range(8 * half, 8 * half + 8):
                    for it in range(2):
                        is_stop = jt == 15 and (h % 8) == 7
                        mm = nc.tensor.matmul(
                            ctx_ps[:, it * DIM + h * DK : it * DIM + (h + 1) * DK],
                            attn[:, slot_col(h) + it * 128 : slot_col(h) + (it + 1) * 128],
                            Vh_sb[:, jt * DIM + h * DK : jt * DIM + (h + 1) * DK],
                            start=False, stop=is_stop,
                            skip_group_check=True,
                        )
                        if h % 8 == 0 and it == 0 and (jt + 1) in sc_last:
                            add_dep_helper(
                                mm.ins, sc_last[jt + 1].ins, sync=False,
                                reason="scores ahead of ctx on PE",
                            )

            def emit_jt(jt):
                e_sb = attnp.tile([128, 16 * SS], bf16, tag="e", bufs=4)
                for g in range(4):
                    sc_ps = psB.tile([128, 4 * SS], f32, tag="sc", bufs=2)
                    for u in range(2):
                        for par in range(2):
                            h = 4 * g + 2 * u + par
                            t = h // 2
                            sc_last[jt] = nc.tensor.matmul(
                                sc_ps[:, (u if par == 0 else 2 + u) * SS :][:, :SS],
                                KhT_sb[64 * par : 64 * par + 64,
                                       t * SEQ + jt * 128 : t * SEQ + (jt + 1) * 128],
                                QhT_sb[64 * par : 64 * par + 64,
                                       t * SS : (t + 1) * SS],
                                start=True, stop=True,
                            )
                    nc.scalar.activation(
                        e_sb[:, g * 4 * SS : (g + 1) * 4 * SS], sc_ps[:],
                        AF.Exp, bias=zb[:],
                    )
                    # Pool tree (heads 0-7) starts right after exp group 1
                    if g == 1:
                        a1 = attnp.tile([128, 4 * SS], bf16, tag="a1", bufs=2)
                        nc.gpsimd.tensor_add(a1[:], e_sb[:, 0 : 4 * SS],
                                             e_sb[:, 4 * SS : 8 * SS])
                        a2 = attnp.tile([128, 2 * SS], bf16, tag="a2", bufs=2)
                        nc.gpsimd.tensor_add(a2[:], a1[:, 0 : 2 * SS],
                                             a1[:, 2 * SS : 4 * SS])
                        a3 = attnp.tile([128, SS], bf16, tag="a3", bufs=2)
                        nc.gpsimd.tensor_add(a3[:], a2[:, 0:SS], a2[:, SS : 2 * SS])
                # DVE tree (heads 8-15), combine, recip, normalize (2 halves)
                b1 = attnp.tile([128, 4 * SS], bf16, tag="b1", bufs=2)
                nc.vector.tensor_add(b1[:], e_sb[:, 8 * SS : 12 * SS],
                                     e_sb[:, 12 * SS : 16 * SS])
                b2 = attnp.tile([128, 2 * SS], bf16, tag="b2", bufs=2)
                nc.vector.tensor_add(b2[:], b1[:, 0 : 2 * SS], b1[:, 2 * SS : 4 * SS])
                b3 = attnp.tile([128, SS], bf16, tag="b3", bufs=2)
                nc.vector.tensor_add(b3[:], b2[:, 0:SS], b2[:, SS : 2 * SS])
                Dsum = attnp.tile([128, SS], bf16, tag="Dsum", bufs=2)
                nc.vector.tensor_add(Dsum[:], a3[:], b3[:])
                Rf = attnp.tile([128, SS], f32, tag="Rf", bufs=2)
                nc.vector.reciprocal(Rf[:], Dsum[:])
                Rcp = attnp.tile([128, SS], bf16, tag="Rcp", bufs=2)
                nc.vector.tensor_copy(Rcp[:], Rf[:])
                attn = attnp.tile([128, 16 * SS], bf16, tag="attn", bufs=4)
                for half in range(2):
                    nc.vector.tensor_mul(
                        attn[:, half * 8 * SS : (half + 1) * 8 * SS].rearrange(
                            "p (s j) -> p s j", s=8),
                        e_sb[:, half * 8 * SS : (half + 1) * 8 * SS].rearrange(
                            "p (s j) -> p s j", s=8),
                        Rcp[:].unsqueeze(1).broadcast_to([128, 8, SS]),
                    )
                attn_q.append((jt, attn))
                if len(attn_q) > 2:
                    j0, a0 = attn_q.pop(0)
                    emit_ctx_half(j0, a0, 0)
                    emit_ctx_half(j0, a0, 1)

            emit_jt(0)

            # ctx accumulator: zero the 4 banks via one full-bank
            # start=True matmul each (runs in early-loop PE slack).
            ctx_ps = psB.tile([128, 2 * DIM], f32, tag="ctx", name="ctx_ps")
            for b in range(4):
                nc.tensor.matmul(
                    ctx_ps[:, 512 * b : 512 * (b + 1)],
                    z512[:, 0:128], z512[:, 0:512],
                    start=True, stop=False, skip_group_check=True,
                )

            for jt in range(1, 16):
                emit_jt(jt)
            while attn_q:
                j0, a0 = attn_q.pop(0)
                emit_ctx_half(j0, a0, 0)
                emit_ctx_half(j0, a0, 1)

            # keep the PE p-state warm through the softmax-flush idle so
            # the output projection charges full-speed cycles
            warm2 = psB.tile([128, 4 * SS], f32, tag="sc", bufs=2, name="warm2")
            for _ in range(120):
                nc.tensor.matmul(warm2[0:64, 0:64], ident[:, 0:64],
                                 ident[:, 0:64], start=True, stop=True)

            # ---------------- Phase C: output projection --------------------
            with tc.tile_pool(name="cpool", bufs=1) as cpool:
                ctx_sb = cpool.tile([128, 2 * DIM], bf16)
                ctxT_sb = cpool.tile([128, 2 * DIM], bf16)
                # per-bank ctx copies (split ACT/DVE), each chased by a
                # 512-wide xbar DMA transpose into the d-tile layout
                # ctxT[(dt) d, it*128 + i]. No PE or PSUM involved.
                ctxT_v = ctxT_sb[:].rearrange(
                    "p (dt itt i) -> p dt itt i", dt=8, itt=2)
                for idx, (it, bh) in enumerate(
                        ((0, 0), (1, 0), (0, 1), (1, 1))):
                    b = it * 2 + bh
                    src = ctx_ps[:, b * 512 : (b + 1) * 512]
                    dst = ctx_sb[:, b * 512 : (b + 1) * 512]
                    if idx % 2 == 0:
                        nc.scalar.activation(dst, src, AF.Copy)
                    else:
                        nc.vector.tensor_copy(dst, src)
                    nc.sync.dma_start_transpose(
                        ctxT_v[:, 4 * bh : 4 * bh + 4, it, :], dst)
                # O = ctx @ Wo^T + bo : out[(it) i, n], N=512 per matmul;
                # output copied (ACT) and stored in 512-wide chunks on
                # alternating DMA rings (SP / ACT).
                out_sb = cpool.tile([128, 2 * DIM], f32)
                # the 4 output-projection groups alternate between the two
                # sc-tag tiles so a group's start never waits on the
                # previous group's PSUM reader in the same tile
                ops_tiles = [
                    psB.tile([128, 4 * SS], f32, tag="sc", bufs=2,
                             name=f"ops{i}")
                    for i in range(2)
                ]
                for it in range(2):
                    for nh in range(2):
                        g = 2 * it + nh
                        ops = ops_tiles[g % 2][:, (g // 2) * 512 :][:, 0:512]
                        for kt in range(8):
                            nc.tensor.matmul(
                                ops,
                                ctxT_sb[:, kt * SS + it * 128 : kt * SS + (it + 1) * 128],
                                WoT_sb[:, kt * DIM + nh * 512 : kt * DIM + (nh + 1) * 512],
                                start=(kt == 0), stop=False,
                                skip_group_check=True,
                            )
                        nc.tensor.matmul(
                            ops, ones[:, 0:128],
                            bo_sb[:, nh * 512 : (nh + 1) * 512],
                            start=False, stop=True,
                            skip_group_check=True,
                        )
                        osl = slice(it * DIM + nh * 512, it * DIM + (nh + 1) * 512)
                        nc.scalar.activation(out_sb[:, osl], ops, AF.Copy)
                        dma_eng = nc.sync if g % 2 == 0 else nc.scalar
                        dma_eng.dma_start(
                            out.ap().rearrange(
                                "(mt p) (nh d) -> p mt nh d", p=128, nh=2
                            )[:, it, nh, :],
                            out_sb[:, osl],
                        )


def get_nc():
    if "nc" not in _CACHE:
        _CACHE["nc"] = _build()
    return _CACHE["nc"]


def make_in_maps(inputs):
    f = lambda x: np.ascontiguousarray(np.asarray(x, dtype=np.float32))
    bf = ml_dtypes.bfloat16
    q, k, v = f(inputs["q"]), f(inputs["k"]), f(inputs["v"])
    WqTs = np.ascontiguousarray((f(inputs["Wq"]) * SCALE).T.astype(bf))
    WkT = np.ascontiguousarray(f(inputs["Wk"]).T.astype(bf))
    WvT = np.ascontiguousarray(f(inputs["Wv"]).T.astype(bf))
    WoT = np.ascontiguousarray(f(inputs["Wo"]).T.astype(bf))
    bqs = f(inputs["bq"]) * np.float32(SCALE)
    bk, bv, bo = f(inputs["bk"]), f(inputs["bv"]), f(inputs["bo"])
    in_maps = []
    for c in range(NCORES):
        sl = slice(c * SS, (c + 1) * SS)
        in_maps.append({
            "qT": np.ascontiguousarray(q[sl].T.astype(bf)),
            "kT": np.ascontiguousarray(k[sl].T.astype(bf)),
            "vT": np.ascontiguousarray(v[sl].T.astype(bf)),
            "WqT": WqTs, "WkT": WkT, "WvT": WvT, "WoT": WoT,
            "bq": bqs, "bk": bk, "bv": bv, "bo": bo,
        })
    return in_maps


def run(inputs, **kwargs):
    """Run on hardware; returns (output, BassKernelResults)."""
    from concourse import bass_utils

    nc = get_nc()
    res = bass_utils.run_bass_kernel_spmd(
        nc, make_in_maps(inputs), core_ids=list(range(NCORES)), **kwargs
    )
    rows = [res.results[c]["out"] for c in range(NCORES)]
    full = np.concatenate(rows, axis=0).astype(np.float32)
    return full.reshape(1, SEQ, DIM), res


def kernel(**inputs) -> np.ndarray:
    out, _ = run(inputs)
    return out
